# revision 4
# baseline (speedup 1.0000x reference)
"""CRF negative log-likelihood loss on 8 Trainium2 NeuronCores.

Strategy: data-parallel over batch (64 sequences per core) with a
meet-in-the-middle split of each sequence: a forward recurrence over steps
1..512 and a backward recurrence over steps 1024..513 run concurrently,
packed into one 98-partition datapath (rows 0-48: fwd = 48 real states +
hold; rows 49-97: bwd = 48 real states + src). Per device step: one PE
matmul with a block-diagonal [98,98] bf16 stationary + one DVE Hadamard
with pre-exponentiated bf16 emissions. This halves the serial depth of the
scan (512 instead of 1025 dependent matmul->mul pairs).

Variable lengths: batches with L<=511 finish inside the fwd half; a "hold"
state absorbs the terminal score at tau=L+1 (score = log alpha_hold +
C_at_L). For L>=513 the bwd chain reconstructs beta_512 starting from a
"src" state that injects g_L * exp(trans[STOP,:]) at device step 1024-L
(the device primitive is u' = g (x) (M u), so the reversed emission stream
is shifted by one and the last step applies bare A). The partition function
is then the host-side dot alpha_512 . beta_512. L=512 uses a host dot with
exp(trans[STOP,:]). All work runs in the exponential domain with
host-precomputed per-(batch,step) shifts keeping magnitudes in range; the
gold path score is a cheap O(B*T) host gather.
"""
import numpy as np
import ml_dtypes
from contextlib import ExitStack

import concourse.bacc as bacc
import concourse.bass as bass
import concourse.tile as tile
from concourse import mybir
from concourse.bass_utils import run_bass_kernel_spmd

B, T, K = 512, 1024, 48
START, STOP = 46, 47
NEG = -10000.0
KF = 49          # fwd rows: 48 real states + hold
HOLD = 48
SRC = 48         # bwd-local index of src state
K2 = 98          # packed partition count
NCORES = 8
BC = B // NCORES  # 64
HC = BC // 2      # 32 columns per chain
S = 512           # device steps
MID = 512         # meeting point
CH = 64           # steps per emission chunk

_nc_cache = {}


def _build_module(s_steps=S, ch=CH):
    key = ("nc", s_steps, ch)
    if key in _nc_cache:
        return _nc_cache[key]
    nc = bacc.Bacc(
        "TRN2",
        target_bir_lowering=False,
        debug=False,
        enable_asserts=False,
        num_devices=NCORES,
    )
    f32 = mybir.dt.float32
    bf16 = mybir.dt.bfloat16
    e_dram = nc.dram_tensor("etil", [K2, K2], bf16, kind="ExternalInput").ap()
    g_dram = nc.dram_tensor("emis", [K2, s_steps, BC], bf16, kind="ExternalInput").ap()
    w0_dram = nc.dram_tensor("w0", [K2, BC], bf16, kind="ExternalInput").ap()
    o_dram = nc.dram_tensor("uout", [K2, BC], f32, kind="ExternalOutput").ap()

    with tile.TileContext(nc) as tc:
        with ExitStack() as ctx:
            const = ctx.enter_context(tc.tile_pool(name="const", bufs=1))
            wpool = ctx.enter_context(tc.tile_pool(name="wp", bufs=4))
            gexp_p = ctx.enter_context(tc.tile_pool(name="gexp", bufs=2))
            psum_p = ctx.enter_context(tc.tile_pool(name="ps", bufs=4, space="PSUM"))

            etile = const.tile([K2, K2], bf16)
            nc.sync.dma_start(out=etile, in_=e_dram)

            wa = const.tile([K2, HC], bf16)
            nc.sync.dma_start(out=wa, in_=w0_dram[:, 0:HC])
            wb = const.tile([K2, HC], bf16)
            nc.sync.dma_start(out=wb, in_=w0_dram[:, HC:BC])

            outa = const.tile([K2, HC], f32)
            outb = const.tile([K2, HC], f32)

            nstep = 0
            while nstep < s_steps:
                ns = min(ch, s_steps - nstep)
                gexp = gexp_p.tile([K2, ch, BC], bf16, tag="gexp")
                nc.sync.dma_start(
                    out=gexp[:, :ns, :], in_=g_dram[:, nstep : nstep + ns, :]
                )
                for s in range(ns):
                    last = nstep + s == s_steps - 1
                    psa = psum_p.tile([K2, HC], f32, tag="psa")
                    nc.tensor.matmul(psa, etile, wa, start=True, stop=True)
                    psb = psum_p.tile([K2, HC], f32, tag="psb")
                    nc.tensor.matmul(psb, etile, wb, start=True, stop=True)
                    if last:
                        nc.vector.tensor_mul(outa, psa, gexp[:, s, 0:HC])
                        nc.vector.tensor_mul(outb, psb, gexp[:, s, HC:BC])
                    else:
                        wa2 = wpool.tile([K2, HC], bf16, tag="wa")
                        nc.vector.tensor_mul(wa2, psa, gexp[:, s, 0:HC])
                        wa = wa2
                        wb2 = wpool.tile([K2, HC], bf16, tag="wb")
                        nc.vector.tensor_mul(wb2, psb, gexp[:, s, HC:BC])
                        wb = wb2
                nstep += ns
            nc.sync.dma_start(out=o_dram[:, 0:HC], in_=outa)
            nc.sync.dma_start(out=o_dram[:, HC:BC], in_=outb)

    nc.compile()
    _nc_cache[key] = nc
    return nc


def _host_prep(feats, seq_len, trans):
    """Build per-core packed emission tensors [K2, S, BC] (bf16), init
    vectors, the block stationary, and shift-correction context."""
    feats = np.ascontiguousarray(feats, dtype=np.float32)
    seq_len = np.asarray(seq_len, dtype=np.int64)
    trans = np.asarray(trans, dtype=np.float32)

    mx = feats.max(axis=2)  # [B, T]
    E64 = np.exp(trans.astype(np.float64)).T  # E[p,n]

    # fwd drift calibration
    drift = []
    for b in range(6):
        fv = np.full(K, NEG, dtype=np.float64)
        fv[START] = 0.0
        Lb = int(seq_len[b])
        for t in range(min(Lb, 256)):
            m = fv.max()
            wv = np.exp(fv - m)
            with np.errstate(divide="ignore"):
                fv = np.log(E64.T @ wv) + m + feats[b, t]
            drift.append((fv.max() - m) - mx[b, t])
    mu = float(np.mean(drift))

    # bwd drift calibration
    driftb = []
    nb = 0
    for b in range(B):
        Lb = int(seq_len[b])
        if Lb < 700:
            continue
        nb += 1
        if nb > 6:
            break
        bv = trans[STOP, :].astype(np.float64).copy()
        for t in range(Lb, max(Lb - 256, MID), -1):
            m = bv.max()
            wv = np.exp(bv - m)
            gv = np.exp(feats[b, t - 1].astype(np.float64))
            with np.errstate(divide="ignore"):
                bv = np.log(E64 @ (gv * wv)) + m
            driftb.append((bv.max() - m) - mx[b, t - 1])
    mub = float(np.mean(driftb)) if driftb else mu

    c = mx + mu
    Ccum = np.cumsum(c, axis=1, dtype=np.float64)
    C_at_L = Ccum[np.arange(B), seq_len - 1]
    Cf = Ccum[:, MID - 1]
    cb = mx + mub
    Ccumb = np.cumsum(cb, axis=1, dtype=np.float64)
    Cb = Ccumb[np.arange(B), seq_len - 1] - Ccumb[:, MID - 1]

    taus = np.arange(1, S + 1)
    g = np.zeros((B, S, K2), dtype=np.float32)
    # fwd real emissions
    livef = taus[None, :] <= seq_len[:, None]
    gf = np.exp(feats[:, :S, :] - c[:, :S, None])
    g[:, :, :K] = np.where(livef[:, :, None], gf, 0.0)
    holdon = taus[None, :] >= (seq_len[:, None] + 1)
    g[:, :, HOLD] = np.where(holdon, 1.0, 0.0)
    # bwd emissions (shifted-reversed stream; see module docstring)
    tau_e = 1024 - taus
    vlong = seq_len >= MID + 1
    liveb = (
        (tau_e[None, :] <= seq_len[:, None])
        & (tau_e[None, :] >= MID + 1)
        & vlong[:, None]
    )
    gb = np.exp(feats[:, ::-1, :][:, :S, :] - cb[:, ::-1][:, :S, None])
    g[:, :, KF : KF + K] = np.where(liveb[:, :, None], gb, 0.0)
    g[vlong, S - 1, KF : KF + K] = 1.0
    srcon = (tau_e[None, :] > seq_len[:, None]) & vlong[:, None]
    g[:, :, KF + SRC] = np.where(srcon, 1.0, 0.0)

    # init u0 [B, K2]
    u0 = np.zeros((B, K2), dtype=np.float32)
    u0[:, START] = 1.0
    isT = seq_len == T
    u0[isT, KF : KF + K] = np.exp(
        feats[isT, T - 1, :] - cb[isT, T - 1][:, None]
    ) * np.exp(trans[STOP, :])[None, :]
    u0[(seq_len >= MID + 1) & (seq_len < T), KF + SRC] = 1.0

    gq = g.astype(ml_dtypes.bfloat16)
    per_core_g = []
    per_core_u0 = []
    for cix in range(NCORES):
        blk = gq[cix * BC : (cix + 1) * BC]            # [BC, S, K2]
        per_core_g.append(np.ascontiguousarray(blk.transpose(2, 1, 0)))
        per_core_u0.append(
            np.ascontiguousarray(
                u0[cix * BC : (cix + 1) * BC].T.astype(ml_dtypes.bfloat16)
            )
        )

    etil2 = np.zeros((K2, K2), dtype=np.float32)
    etil2[:K, :K] = np.exp(trans).T
    etil2[:K, HOLD] = np.exp(trans[STOP, :])
    etil2[HOLD, HOLD] = 1.0
    etil2[KF : KF + K, KF : KF + K] = np.exp(trans)
    etil2[KF + SRC, KF : KF + K] = np.exp(trans[STOP, :])
    etil2[KF + SRC, KF + SRC] = 1.0
    etil2 = etil2.astype(ml_dtypes.bfloat16)

    ctx = {
        "C_at_L": C_at_L,
        "Cf": Cf,
        "Cb": Cb,
        "seq_len": seq_len,
        "estop": np.exp(trans[STOP, :K].astype(np.float64)),
    }
    return per_core_g, per_core_u0, etil2, ctx


def _combine(uout, ctx):
    """uout: [K2, B] f64 device outputs; returns per-batch forward scores."""
    seq_len = ctx["seq_len"]
    alpha = uout[:KF, :]
    beta = uout[KF:, :]
    scores = np.zeros(B)
    short = seq_len <= MID - 1
    scores[short] = np.log(alpha[HOLD, short]) + ctx["C_at_L"][short]
    isM = seq_len == MID
    if isM.any():
        dotM = (alpha[:K, :] * ctx["estop"][:, None]).sum(axis=0)
        scores[isM] = np.log(dotM[isM]) + ctx["Cf"][isM]
    vlong = seq_len >= MID + 1
    dot = (alpha[:K, :] * beta[:K, :]).sum(axis=0)
    scores[vlong] = np.log(dot[vlong]) + ctx["Cf"][vlong] + ctx["Cb"][vlong]
    return scores


def _gold_score(feats, tags, seq_len, trans):
    feats = np.asarray(feats, dtype=np.float32)
    tags = np.asarray(tags, dtype=np.int64)
    seq_len = np.asarray(seq_len, dtype=np.int64)
    trans = np.asarray(trans, dtype=np.float32)
    tags_ext = np.concatenate(
        [np.full((B, 1), START, dtype=np.int64), tags], axis=1
    )
    trans_sc = trans[tags_ext[:, 1:], tags_ext[:, :-1]]
    emit_sc = np.take_along_axis(feats, tags_ext[:, 1:, None], axis=2)[..., 0]
    mask = np.arange(T)[None, :] < seq_len[:, None]
    last_tag = np.take_along_axis(tags_ext, seq_len[:, None], axis=1)[:, 0]
    gold = (
        np.where(mask, trans_sc + emit_sc, 0.0).sum(1, dtype=np.float64)
        + trans[STOP, last_tag]
    )
    return gold  # [B] f64


def kernel(feats, tags, seq_len, transitions):
    feats = np.asarray(feats)
    per_core_g, per_core_u0, etil2, ctx = _host_prep(feats, seq_len, transitions)
    nc = _build_module()
    in_maps = [
        {"etil": etil2, "emis": per_core_g[c], "w0": per_core_u0[c]}
        for c in range(NCORES)
    ]
    res = run_bass_kernel_spmd(nc, in_maps, list(range(NCORES)))
    uout = np.concatenate(
        [np.asarray(res.results[c]["uout"]).astype(np.float64) for c in range(NCORES)],
        axis=1,
    )  # [K2, B]
    scores = _combine(uout, ctx)
    gold = _gold_score(feats, tags, seq_len, transitions)
    loss = np.mean(scores - gold)
    return np.float32(loss)


# revision 5
# speedup vs baseline: 1.2810x; 1.2810x over previous
"""CRF negative log-likelihood loss on 8 Trainium2 NeuronCores.

Strategy: data-parallel over batch (64 sequences per core) with a 3-segment
split of each sequence that cuts the serial scan depth to 342 device slots:

  seg1 [tau 1..342]:    exact forward chain "alpha" (48 states + hold)
  seg2 [tau 343..683]:  two seeded chains through the middle transport Phi:
                        x = A.Phi.1 (fwd-seeded) and y = Phi^T.e_stop
                        (bwd-seeded). Products of 341 positive transfer
                        matrices are numerically rank-1 (Birkhoff
                        contraction), so Phi alpha_342 ~ x (y.alpha)/(y.1).
  seg3 [tau 684..1024]: exact backward chain "b" = G_684 beta_684
                        (48 states + src injection at tau=L).

Partition packing: pack1 rows = [alpha | y], pack2 rows = [x | b], each a
[98, 64] datapath sharing one block [98,98] bf16 stationary. Per slot each
pack does one PE matmul + one DVE Hadamard with pre-exponentiated bf16
emissions; the two packs are independent chains that interleave on the
engines, hiding the cross-engine latency. PSUM accumulation stays fp32.

Variable lengths, per batch: L<=341 resolves via the fwd hold state;
L=342 via a host dot with exp(trans[STOP,:]); 343<=L<=683 via the exact
y-chain (src-injected at tau=L): Z = y . alpha_342; L>=684 via the rank-1
composition Z ~ (b.x)(y.alpha_342)/(y.1). All chains run in the
exponential domain with host-precomputed per-(batch,step) shifts; the gold
path score is a cheap O(B*T) host gather.
"""
import numpy as np
import ml_dtypes
from contextlib import ExitStack

import concourse.bacc as bacc
import concourse.bass as bass
import concourse.tile as tile
from concourse import mybir
from concourse.bass_utils import run_bass_kernel_spmd

B, T, K = 512, 1024, 48
START, STOP = 46, 47
NEG = -10000.0
HOLD = 48
SRCL = 48         # src local index within bwd half
KF = 49
K2 = 98
NCORES = 8
BC = B // NCORES  # 64
D = 342           # device slots
M1 = 342          # end of seg1
M2 = 683          # end of seg2
CH = 57           # slots per emission chunk (342 = 6*57)

_nc_cache = {}


def _build_module(d_slots=D, ch=CH):
    key = ("nc", d_slots, ch)
    if key in _nc_cache:
        return _nc_cache[key]
    nc = bacc.Bacc(
        "TRN2",
        target_bir_lowering=False,
        debug=False,
        enable_asserts=False,
        num_devices=NCORES,
    )
    f32 = mybir.dt.float32
    bf16 = mybir.dt.bfloat16
    e_dram = nc.dram_tensor("etil", [K2, K2], bf16, kind="ExternalInput").ap()
    g1_dram = nc.dram_tensor("emis1", [K2, d_slots, BC], bf16, kind="ExternalInput").ap()
    g2_dram = nc.dram_tensor("emis2", [K2, d_slots, BC], bf16, kind="ExternalInput").ap()
    w0_dram = nc.dram_tensor("w0", [K2, 2 * BC], bf16, kind="ExternalInput").ap()
    o_dram = nc.dram_tensor("uout", [K2, 2 * BC], f32, kind="ExternalOutput").ap()

    with tile.TileContext(nc) as tc:
        with ExitStack() as ctx:
            const = ctx.enter_context(tc.tile_pool(name="const", bufs=1))
            wpool = ctx.enter_context(tc.tile_pool(name="wp", bufs=4))
            gexp_p = ctx.enter_context(tc.tile_pool(name="gexp", bufs=2))
            psum_p = ctx.enter_context(tc.tile_pool(name="ps", bufs=4, space="PSUM"))

            etile = const.tile([K2, K2], bf16)
            nc.sync.dma_start(out=etile, in_=e_dram)

            w1 = const.tile([K2, BC], bf16)
            nc.sync.dma_start(out=w1, in_=w0_dram[:, 0:BC])
            w2 = const.tile([K2, BC], bf16)
            nc.sync.dma_start(out=w2, in_=w0_dram[:, BC : 2 * BC])

            out1 = const.tile([K2, BC], f32)
            out2 = const.tile([K2, BC], f32)

            nstep = 0
            while nstep < d_slots:
                ns = min(ch, d_slots - nstep)
                ge1 = gexp_p.tile([K2, ch, BC], bf16, tag="ge1")
                nc.sync.dma_start(
                    out=ge1[:, :ns, :], in_=g1_dram[:, nstep : nstep + ns, :]
                )
                ge2 = gexp_p.tile([K2, ch, BC], bf16, tag="ge2")
                nc.sync.dma_start(
                    out=ge2[:, :ns, :], in_=g2_dram[:, nstep : nstep + ns, :]
                )
                for s in range(ns):
                    last = nstep + s == d_slots - 1
                    ps1 = psum_p.tile([K2, BC], f32, tag="ps1")
                    nc.tensor.matmul(ps1, etile, w1, start=True, stop=True)
                    ps2 = psum_p.tile([K2, BC], f32, tag="ps2")
                    nc.tensor.matmul(ps2, etile, w2, start=True, stop=True)
                    if last:
                        nc.vector.tensor_mul(out1, ps1, ge1[:, s, :])
                        nc.vector.tensor_mul(out2, ps2, ge2[:, s, :])
                    else:
                        w1n = wpool.tile([K2, BC], bf16, tag="w1")
                        nc.vector.tensor_mul(w1n, ps1, ge1[:, s, :])
                        w1 = w1n
                        w2n = wpool.tile([K2, BC], bf16, tag="w2")
                        nc.vector.tensor_mul(w2n, ps2, ge2[:, s, :])
                        w2 = w2n
                nstep += ns
            nc.sync.dma_start(out=o_dram[:, 0:BC], in_=out1)
            nc.sync.dma_start(out=o_dram[:, BC : 2 * BC], in_=out2)

    nc.compile()
    _nc_cache[key] = nc
    return nc


def _host_prep(feats, seq_len, trans):
    feats = np.ascontiguousarray(feats, dtype=np.float32)
    seq_len = np.asarray(seq_len, dtype=np.int64)
    trans = np.asarray(trans, dtype=np.float32)
    L = seq_len

    mx = feats.max(axis=2)
    E64 = np.exp(trans.astype(np.float64)).T

    # fwd drift calibration
    drift = []
    for b in range(6):
        fv = np.full(K, NEG, dtype=np.float64)
        fv[START] = 0.0
        Lb = int(L[b])
        for t in range(min(Lb, 256)):
            m = fv.max()
            wv = np.exp(fv - m)
            with np.errstate(divide="ignore"):
                fv = np.log(E64.T @ wv) + m + feats[b, t]
            drift.append((fv.max() - m) - mx[b, t])
    mu = float(np.mean(drift))

    # bwd drift calibration
    driftb = []
    nb = 0
    for b in range(B):
        Lb = int(L[b])
        if Lb < 700:
            continue
        nb += 1
        if nb > 6:
            break
        bv = trans[STOP, :].astype(np.float64).copy()
        for t in range(Lb, max(Lb - 256, M1), -1):
            m = bv.max()
            wv = np.exp(bv - m)
            gv = np.exp(feats[b, t - 1].astype(np.float64))
            with np.errstate(divide="ignore"):
                bv = np.log(E64 @ (gv * wv)) + m
            driftb.append((bv.max() - m) - mx[b, t - 1])
    mub = float(np.mean(driftb)) if driftb else mu

    c = mx + mu
    cb = mx + mub
    Ccum = np.cumsum(c, axis=1, dtype=np.float64)
    Ccumb = np.cumsum(cb, axis=1, dtype=np.float64)

    estop = np.exp(trans[STOP, :K].astype(np.float64))
    n_ = np.arange(1, D + 1)

    # ---- pack1 emissions [B, D, K2] ----
    g1 = np.zeros((B, D, K2), dtype=np.float32)
    livef = n_[None, :] <= L[:, None]
    gf = np.exp(feats[:, :D, :] - c[:, :D, None])
    g1[:, :, :K] = np.where(livef[:, :, None], gf, 0.0)
    holdon = n_[None, :] >= (L[:, None] + 1)
    g1[:, :, HOLD] = np.where(holdon, 1.0, 0.0)

    gyb = np.exp(feats[:, ::-1, :] - cb[:, ::-1][:, :, None])  # rev idx r -> tau=1024-r
    yact = L >= M1 + 1
    sstar_y = np.where(L >= M2 + 1, 1, M2 + 1 - L)
    tau_y = M2 + 1 - n_
    ridx = 1024 - tau_y
    valid = (n_[None, :] >= sstar_y[:, None]) & (n_[None, :] <= D - 1) & yact[:, None]
    gy = gyb[:, np.clip(ridx, 0, 1023), :]
    g1[:, :, KF : KF + K] = np.where(valid[:, :, None], gy, 0.0)
    g1[yact, D - 1, KF : KF + K] = 1.0
    srcon_y = (n_[None, :] < sstar_y[:, None]) & yact[:, None]
    g1[:, :, KF + SRCL] = np.where(srcon_y, 1.0, 0.0)

    # ---- pack2 emissions ----
    g2 = np.zeros((B, D, K2), dtype=np.float32)
    xact = L >= M2 + 1
    tau_x = M1 + n_
    validx = (n_[None, :] <= D - 1) & xact[:, None]
    gxv = np.exp(feats - c[:, :, None])
    gx = gxv[:, np.clip(tau_x - 1, 0, 1023), :]
    g2[:, :, :K] = np.where(validx[:, :, None], gx, 0.0)
    g2[xact, D - 1, :K] = 1.0
    sstar_b = 1026 - L
    tau_b = 1026 - n_
    ridx_b = 1024 - tau_b
    validb = (n_[None, :] >= sstar_b[:, None]) & xact[:, None]
    gbv = gyb[:, np.clip(ridx_b, 0, 1023), :]
    g2[:, :, KF : KF + K] = np.where(validb[:, :, None], gbv, 0.0)
    srcon_b = (n_[None, :] < sstar_b[:, None]) & xact[:, None]
    g2[:, :, KF + SRCL] = np.where(srcon_b, 1.0, 0.0)

    # ---- inits [B, K2] ----
    u1 = np.zeros((B, K2), dtype=np.float32)
    u1[:, START] = 1.0
    u1[yact, KF + SRCL] = 1.0
    u2 = np.zeros((B, K2), dtype=np.float32)
    u2[xact, :K] = 1.0
    u2[xact, KF + SRCL] = 1.0

    # ---- per-core slices ----
    g1q = g1.astype(ml_dtypes.bfloat16)
    g2q = g2.astype(ml_dtypes.bfloat16)
    per_core = []
    for cix in range(NCORES):
        sl = slice(cix * BC, (cix + 1) * BC)
        pg1 = np.ascontiguousarray(g1q[sl].transpose(2, 1, 0))
        pg2 = np.ascontiguousarray(g2q[sl].transpose(2, 1, 0))
        w0 = np.concatenate([u1[sl].T, u2[sl].T], axis=1).astype(ml_dtypes.bfloat16)
        per_core.append({"emis1": pg1, "emis2": pg2, "w0": np.ascontiguousarray(w0)})

    S_ = np.zeros((K2, K2), dtype=np.float32)
    S_[:K, :K] = np.exp(trans).T
    S_[:K, HOLD] = estop.astype(np.float32)
    S_[HOLD, HOLD] = 1.0
    S_[KF : KF + K, KF : KF + K] = np.exp(trans)
    S_[KF + SRCL, KF : KF + K] = estop.astype(np.float32)
    S_[KF + SRCL, KF + SRCL] = 1.0
    etil = S_.astype(ml_dtypes.bfloat16)

    ctx = {
        "seq_len": L,
        "estop": estop,
        "C_at_L": Ccum[np.arange(B), L - 1],
        "Calpha": Ccum[:, M1 - 1],
        "Cx": Ccum[:, M2 - 1] - Ccum[:, M1 - 1],
        "Cy": Ccumb[np.arange(B), L - 1] - Ccumb[:, M1 - 1],
        "Cb": Ccumb[np.arange(B), L - 1] - Ccumb[:, M2 - 1],
    }
    return per_core, etil, ctx


def _combine(uout, ctx):
    """uout: [K2, 2B] f64 device outputs (pack1 | pack2 per core already
    re-assembled into [K2, B] pairs); returns per-batch forward scores."""
    p1, p2 = uout
    L = ctx["seq_len"]
    alpha = p1[:KF, :]
    y = p1[KF : KF + K, :]
    x = p2[:K, :]
    bv = p2[KF : KF + K, :]
    estop = ctx["estop"]

    scores = np.zeros(B)
    short = L <= M1 - 1
    scores[short] = np.log(alpha[HOLD, short]) + ctx["C_at_L"][short]
    isM = L == M1
    if isM.any():
        dotM = (alpha[:K, :] * estop[:, None]).sum(0)
        scores[isM] = np.log(dotM[isM]) + ctx["Calpha"][isM]
    midL = (L >= M1 + 1) & (L <= M2)
    dya = (y * alpha[:K, :]).sum(0)
    scores[midL] = np.log(dya[midL]) + ctx["Calpha"][midL] + ctx["Cy"][midL]
    lng = L >= M2 + 1
    dbx = (bv * x).sum(0)
    y1 = y.sum(0)
    scores[lng] = (
        np.log(dbx[lng])
        + np.log(dya[lng])
        - np.log(y1[lng])
        + ctx["Calpha"][lng]
        + ctx["Cx"][lng]
        + ctx["Cb"][lng]
    )
    return scores


def _gold_score(feats, tags, seq_len, trans):
    feats = np.asarray(feats, dtype=np.float32)
    tags = np.asarray(tags, dtype=np.int64)
    seq_len = np.asarray(seq_len, dtype=np.int64)
    trans = np.asarray(trans, dtype=np.float32)
    tags_ext = np.concatenate(
        [np.full((B, 1), START, dtype=np.int64), tags], axis=1
    )
    trans_sc = trans[tags_ext[:, 1:], tags_ext[:, :-1]]
    emit_sc = np.take_along_axis(feats, tags_ext[:, 1:, None], axis=2)[..., 0]
    mask = np.arange(T)[None, :] < seq_len[:, None]
    last_tag = np.take_along_axis(tags_ext, seq_len[:, None], axis=1)[:, 0]
    gold = (
        np.where(mask, trans_sc + emit_sc, 0.0).sum(1, dtype=np.float64)
        + trans[STOP, last_tag]
    )
    return gold  # [B] f64


def kernel(feats, tags, seq_len, transitions):
    feats = np.asarray(feats)
    per_core, etil, ctx = _host_prep(feats, seq_len, transitions)
    nc = _build_module()
    in_maps = [{"etil": etil, **per_core[c]} for c in range(NCORES)]
    res = run_bass_kernel_spmd(nc, in_maps, list(range(NCORES)))
    outs = [np.asarray(res.results[c]["uout"]).astype(np.float64) for c in range(NCORES)]
    p1 = np.concatenate([o[:, 0:BC] for o in outs], axis=1)   # [K2, B]
    p2 = np.concatenate([o[:, BC : 2 * BC] for o in outs], axis=1)
    scores = _combine((p1, p2), ctx)
    gold = _gold_score(feats, tags, seq_len, transitions)
    loss = np.mean(scores - gold)
    return np.float32(loss)


# revision 6
# speedup vs baseline: 1.4288x; 1.1154x over previous
"""CRF negative log-likelihood loss on 8 Trainium2 NeuronCores.

Strategy: data-parallel over batch (64 sequences per core) with a 5-segment
split of each sequence cutting the serial scan depth to 205 device slots.
Segments (tau ranges): [1,205] [206,410] [411,615] [616,820] [821,1024].

  alpha: exact forward chain over seg1 (48 states + hold for L<=204)
  x_j = T_j . 1         (fwd-seeded through middle segment j = 2,3,4)
  y_j = T_j^T . e_stop  (bwd-seeded, also serves as the exact tail for
                         batches whose L falls inside segment j via a src
                         state injected at tau=L)
  b = A^T G_821 beta_821 (exact backward chain over seg5, src-injected)

Products of 205 positive transfer matrices are numerically rank-1
(Birkhoff contraction), so T_j w ~ x_j (y_j.w)/(y_j.1) and the partition
function composes from host-side dots:
  Z ~ (b.x4)(y4.x3)(y3.x2)(y2.alpha) / [(y2.1)(y3.1)(y4.1)]
truncated at the segment containing L.

Packing: 8 half-chains -> 4 packs of [98, 64] sharing one block [98,98]
bf16 stationary: P1=[alpha|y2] P2=[x2|y3] P3=[x3|y4] P4=[x4|b]. Packs are
PAIRED: (P1,P2) and (P3,P4) each write one [98,128] PSUM tile (two matmuls
into disjoint column slices) consumed by ONE wide DVE Hadamard, amortizing
the fixed PSUM-access cost; the two pairs are independent chains that
interleave on the engines to hide cross-engine sync latency. All chains
run in the exponential domain (bf16, fp32 PSUM) with host-precomputed
per-(batch,step) shifts; the gold path score is a host gather.
"""
import numpy as np
import ml_dtypes
from contextlib import ExitStack

import concourse.bacc as bacc
import concourse.bass as bass
import concourse.tile as tile
from concourse import mybir
from concourse.bass_utils import run_bass_kernel_spmd

B, T, K = 512, 1024, 48
START, STOP = 46, 47
NEG = -10000.0
HOLD = 48
SRCL = 48
KF = 49
K2 = 98
NCORES = 8
BC = B // NCORES   # 64
BC2 = 2 * BC       # 128 (pair width)
D = 205            # device slots
BND = [0, 205, 410, 615, 820, 1024]
CH = 41            # slots per emission chunk (205 = 5*41)

_nc_cache = {}


def _build_module(d_slots=D, ch=CH):
    key = ("nc", d_slots, ch)
    if key in _nc_cache:
        return _nc_cache[key]
    nc = bacc.Bacc(
        "TRN2",
        target_bir_lowering=False,
        debug=False,
        enable_asserts=False,
        num_devices=NCORES,
    )
    f32 = mybir.dt.float32
    bf16 = mybir.dt.bfloat16
    e_dram = nc.dram_tensor("etil", [K2, K2], bf16, kind="ExternalInput").ap()
    gA_dram = nc.dram_tensor("emis12", [K2, d_slots, BC2], bf16, kind="ExternalInput").ap()
    gB_dram = nc.dram_tensor("emis34", [K2, d_slots, BC2], bf16, kind="ExternalInput").ap()
    w0_dram = nc.dram_tensor("w0", [K2, 2 * BC2], bf16, kind="ExternalInput").ap()
    o_dram = nc.dram_tensor("uout", [K2, 2 * BC2], f32, kind="ExternalOutput").ap()

    with tile.TileContext(nc) as tc:
        with ExitStack() as ctx:
            const = ctx.enter_context(tc.tile_pool(name="const", bufs=1))
            wpool = ctx.enter_context(tc.tile_pool(name="wp", bufs=4))
            gexp_p = ctx.enter_context(tc.tile_pool(name="gexp", bufs=2))
            psum_p = ctx.enter_context(tc.tile_pool(name="ps", bufs=4, space="PSUM"))

            etile = const.tile([K2, K2], bf16)
            nc.sync.dma_start(out=etile, in_=e_dram)

            wA = const.tile([K2, BC2], bf16)
            nc.sync.dma_start(out=wA, in_=w0_dram[:, 0:BC2])
            wB = const.tile([K2, BC2], bf16)
            nc.sync.dma_start(out=wB, in_=w0_dram[:, BC2 : 2 * BC2])

            outA = const.tile([K2, BC2], f32)
            outB = const.tile([K2, BC2], f32)

            nstep = 0
            while nstep < d_slots:
                ns = min(ch, d_slots - nstep)
                geA = gexp_p.tile([K2, ch, BC2], bf16, tag="geA")
                nc.sync.dma_start(
                    out=geA[:, :ns, :], in_=gA_dram[:, nstep : nstep + ns, :]
                )
                geB = gexp_p.tile([K2, ch, BC2], bf16, tag="geB")
                nc.sync.dma_start(
                    out=geB[:, :ns, :], in_=gB_dram[:, nstep : nstep + ns, :]
                )
                for s in range(ns):
                    last = nstep + s == d_slots - 1
                    psA = psum_p.tile([K2, BC2], f32, tag="psA")
                    nc.tensor.matmul(psA[:, 0:BC], etile, wA[:, 0:BC], start=True, stop=True)
                    nc.tensor.matmul(psA[:, BC:BC2], etile, wA[:, BC:BC2], start=True, stop=True)
                    psB = psum_p.tile([K2, BC2], f32, tag="psB")
                    nc.tensor.matmul(psB[:, 0:BC], etile, wB[:, 0:BC], start=True, stop=True)
                    nc.tensor.matmul(psB[:, BC:BC2], etile, wB[:, BC:BC2], start=True, stop=True)
                    if last:
                        nc.vector.tensor_mul(outA, psA, geA[:, s, :])
                        nc.vector.tensor_mul(outB, psB, geB[:, s, :])
                    else:
                        wAn = wpool.tile([K2, BC2], bf16, tag="wA")
                        nc.vector.tensor_mul(wAn, psA, geA[:, s, :])
                        wA = wAn
                        wBn = wpool.tile([K2, BC2], bf16, tag="wB")
                        nc.vector.tensor_mul(wBn, psB, geB[:, s, :])
                        wB = wBn
                nstep += ns
            nc.sync.dma_start(out=o_dram[:, 0:BC2], in_=outA)
            nc.sync.dma_start(out=o_dram[:, BC2 : 2 * BC2], in_=outB)

    nc.compile()
    _nc_cache[key] = nc
    return nc


def _host_prep(feats, seq_len, trans):
    feats = np.ascontiguousarray(feats, dtype=np.float32)
    seq_len = np.asarray(seq_len, dtype=np.int64)
    trans = np.asarray(trans, dtype=np.float32)
    L = seq_len

    mx = feats.max(axis=2)
    E64 = np.exp(trans.astype(np.float64)).T

    drift = []
    for b in range(6):
        fv = np.full(K, NEG, dtype=np.float64)
        fv[START] = 0.0
        Lb = int(L[b])
        for t in range(min(Lb, 256)):
            m = fv.max()
            wv = np.exp(fv - m)
            with np.errstate(divide="ignore"):
                fv = np.log(E64.T @ wv) + m + feats[b, t]
            drift.append((fv.max() - m) - mx[b, t])
    mu = float(np.mean(drift))

    driftb = []
    nb = 0
    for b in range(B):
        Lb = int(L[b])
        if Lb < 700:
            continue
        nb += 1
        if nb > 6:
            break
        bv = trans[STOP, :].astype(np.float64).copy()
        for t in range(Lb, Lb - 200, -1):
            m = bv.max()
            wv = np.exp(bv - m)
            gv = np.exp(feats[b, t - 1].astype(np.float64))
            with np.errstate(divide="ignore"):
                bv = np.log(E64 @ (gv * wv)) + m
            driftb.append((bv.max() - m) - mx[b, t - 1])
    mub = float(np.mean(driftb)) if driftb else mu

    c = mx + mu
    cb = mx + mub
    Ccum = np.cumsum(c, axis=1, dtype=np.float64)
    Ccumb = np.cumsum(cb, axis=1, dtype=np.float64)
    estop = np.exp(trans[STOP, :K].astype(np.float64))
    estop32 = estop.astype(np.float32)

    n_ = np.arange(1, D + 1)
    gfall = np.exp(feats - c[:, :, None])
    gball = np.exp(feats - cb[:, :, None])

    # alpha rows [B, D, KF]
    ga = np.zeros((B, D, KF), dtype=np.float32)
    livef = n_[None, :] <= L[:, None]
    ga[:, :, :K] = np.where(livef[:, :, None], gfall[:, :D, :], 0.0)
    holdon = n_[None, :] >= (L[:, None] + 1)
    ga[:, :, HOLD] = np.where(holdon, 1.0, 0.0)

    def x_rows(j):
        bjm1, bj = BND[j - 1], BND[j]
        act = L > bj
        tau = bjm1 + n_
        g = np.where(act[:, None, None], gfall[:, tau - 1, :], 0.0).astype(np.float32)
        return g, act

    def y_rows(j):
        bjm1, bj = BND[j - 1], BND[j]
        act = L >= bjm1 + 1
        sstar = np.maximum(bj - L, 0)
        g = np.zeros((B, D, KF), dtype=np.float32)
        tau = bj - n_
        valid = (
            (n_[None, :] >= np.maximum(sstar, 1)[:, None])
            & (n_[None, :] <= D - 1)
            & act[:, None]
        )
        gv = gball[:, np.clip(tau - 1, 0, T - 1), :]
        g[:, :, :K] = np.where(valid[:, :, None], gv, 0.0)
        g[act, D - 1, :K] = 1.0
        srcon = (n_[None, :] < sstar[:, None]) & act[:, None]
        g[:, :, SRCL] = np.where(srcon, 1.0, 0.0)
        u0 = np.zeros((B, KF), dtype=np.float32)
        inj = act & (sstar == 0)
        u0[inj, :K] = gball[inj, bj - 1, :] * estop32[None, :]
        u0[act & (sstar > 0), SRCL] = 1.0
        return g, u0

    def b_rows():
        b4 = BND[4]
        act = L >= b4 + 1
        sstar = 1025 - L
        g = np.zeros((B, D, KF), dtype=np.float32)
        tau = 1025 - n_
        valid = (n_[None, :] >= sstar[:, None]) & (n_[None, :] <= D - 1) & act[:, None]
        gv = gball[:, np.clip(tau - 1, 0, T - 1), :]
        g[:, :, :K] = np.where(valid[:, :, None], gv, 0.0)
        g[act, D - 1, :K] = 1.0
        srcon = (n_[None, :] < sstar[:, None]) & act[:, None]
        g[:, :, SRCL] = np.where(srcon, 1.0, 0.0)
        u0 = np.zeros((B, KF), dtype=np.float32)
        u0[act, SRCL] = 1.0
        return g, u0

    gx2, actx2 = x_rows(2)
    gy2, u0y2 = y_rows(2)
    gx3, actx3 = x_rows(3)
    gy3, u0y3 = y_rows(3)
    gx4, actx4 = x_rows(4)
    gy4, u0y4 = y_rows(4)
    gb_, u0b = b_rows()

    def pack(gf, gbk):
        g = np.zeros((B, D, K2), dtype=np.float32)
        g[:, :, : gf.shape[2]] = gf
        g[:, :, KF : KF + gbk.shape[2]] = gbk
        return g

    P1 = pack(ga, gy2)
    P2 = pack(gx2, gy3)
    P3 = pack(gx3, gy4)
    P4 = pack(gx4, gb_)

    u1 = np.zeros((B, K2), dtype=np.float32)
    u1[:, START] = 1.0
    u1[:, KF:] = u0y2
    u2 = np.zeros((B, K2), dtype=np.float32)
    u2[actx2, :K] = 1.0
    u2[:, KF:] = u0y3
    u3 = np.zeros((B, K2), dtype=np.float32)
    u3[actx3, :K] = 1.0
    u3[:, KF:] = u0y4
    u4 = np.zeros((B, K2), dtype=np.float32)
    u4[actx4, :K] = 1.0
    u4[:, KF:] = u0b

    per_core = []
    for cix in range(NCORES):
        sl = slice(cix * BC, (cix + 1) * BC)
        # [K2, D, 128]: cols 0-63 = pack1, 64-127 = pack2 (same sequences)
        gA = np.concatenate(
            [P1[sl].transpose(2, 1, 0), P2[sl].transpose(2, 1, 0)], axis=2
        ).astype(ml_dtypes.bfloat16)
        gB = np.concatenate(
            [P3[sl].transpose(2, 1, 0), P4[sl].transpose(2, 1, 0)], axis=2
        ).astype(ml_dtypes.bfloat16)
        w0 = np.concatenate(
            [u1[sl].T, u2[sl].T, u3[sl].T, u4[sl].T], axis=1
        ).astype(ml_dtypes.bfloat16)
        per_core.append(
            {
                "emis12": np.ascontiguousarray(gA),
                "emis34": np.ascontiguousarray(gB),
                "w0": np.ascontiguousarray(w0),
            }
        )

    S_ = np.zeros((K2, K2), dtype=np.float32)
    S_[:K, :K] = np.exp(trans).T
    S_[:K, HOLD] = estop32
    S_[HOLD, HOLD] = 1.0
    S_[KF : KF + K, KF : KF + K] = np.exp(trans)
    S_[KF + SRCL, KF : KF + K] = estop32
    S_[KF + SRCL, KF + SRCL] = 1.0
    etil = S_.astype(ml_dtypes.bfloat16)

    ar = np.arange(B)
    ctx = {
        "seq_len": L,
        "estop": estop,
        "C_at_L": Ccum[ar, L - 1],
        "Cal": Ccum[:, BND[1] - 1],
        "Cx2": Ccum[:, BND[2] - 1] - Ccum[:, BND[1] - 1],
        "Cx3": Ccum[:, BND[3] - 1] - Ccum[:, BND[2] - 1],
        "Cx4": Ccum[:, BND[4] - 1] - Ccum[:, BND[3] - 1],
        "Cy2": Ccumb[ar, L - 1] - Ccumb[:, BND[1] - 1],
        "Cy3": Ccumb[ar, L - 1] - Ccumb[:, BND[2] - 1],
        "Cy4": Ccumb[ar, L - 1] - Ccumb[:, BND[3] - 1],
        "Cb": Ccumb[ar, L - 1] - Ccumb[:, BND[4] - 1],
    }
    return per_core, etil, ctx


def _combine(packs, ctx):
    """packs: (p1, p2, p3, p4) each [K2, B] f64; returns per-batch scores."""
    p1, p2, p3, p4 = packs
    L = ctx["seq_len"]
    estop = ctx["estop"]
    alpha = p1[:KF, :]
    y2 = p1[KF : KF + K, :]
    x2 = p2[:K, :]
    y3 = p2[KF : KF + K, :]
    x3 = p3[:K, :]
    y4 = p3[KF : KF + K, :]
    x4 = p4[:K, :]
    bv = p4[KF : KF + K, :]

    d_y2a = (y2 * alpha[:K, :]).sum(0)
    d_y3x2 = (y3 * x2).sum(0)
    d_y4x3 = (y4 * x3).sum(0)
    d_bx4 = (bv * x4).sum(0)
    n_y2 = y2.sum(0)
    n_y3 = y3.sum(0)
    n_y4 = y4.sum(0)

    scores = np.zeros(B)
    J1 = L <= BND[1] - 1
    scores[J1] = np.log(alpha[HOLD, J1]) + ctx["C_at_L"][J1]
    JM = L == BND[1]
    if JM.any():
        dm = (alpha[:K, :] * estop[:, None]).sum(0)
        scores[JM] = np.log(dm[JM]) + ctx["Cal"][JM]
    J2 = (L > BND[1]) & (L <= BND[2])
    scores[J2] = np.log(d_y2a[J2]) + ctx["Cal"][J2] + ctx["Cy2"][J2]
    J3 = (L > BND[2]) & (L <= BND[3])
    scores[J3] = (
        np.log(d_y3x2[J3]) + np.log(d_y2a[J3]) - np.log(n_y2[J3])
        + ctx["Cal"][J3] + ctx["Cx2"][J3] + ctx["Cy3"][J3]
    )
    J4 = (L > BND[3]) & (L <= BND[4])
    scores[J4] = (
        np.log(d_y4x3[J4]) + np.log(d_y3x2[J4]) + np.log(d_y2a[J4])
        - np.log(n_y2[J4]) - np.log(n_y3[J4])
        + ctx["Cal"][J4] + ctx["Cx2"][J4] + ctx["Cx3"][J4] + ctx["Cy4"][J4]
    )
    J5 = L > BND[4]
    scores[J5] = (
        np.log(d_bx4[J5]) + np.log(d_y4x3[J5]) + np.log(d_y3x2[J5])
        + np.log(d_y2a[J5]) - np.log(n_y2[J5]) - np.log(n_y3[J5])
        - np.log(n_y4[J5])
        + ctx["Cal"][J5] + ctx["Cx2"][J5] + ctx["Cx3"][J5] + ctx["Cx4"][J5]
        + ctx["Cb"][J5]
    )
    return scores


def _gold_score(feats, tags, seq_len, trans):
    feats = np.asarray(feats, dtype=np.float32)
    tags = np.asarray(tags, dtype=np.int64)
    seq_len = np.asarray(seq_len, dtype=np.int64)
    trans = np.asarray(trans, dtype=np.float32)
    tags_ext = np.concatenate(
        [np.full((B, 1), START, dtype=np.int64), tags], axis=1
    )
    trans_sc = trans[tags_ext[:, 1:], tags_ext[:, :-1]]
    emit_sc = np.take_along_axis(feats, tags_ext[:, 1:, None], axis=2)[..., 0]
    mask = np.arange(T)[None, :] < seq_len[:, None]
    last_tag = np.take_along_axis(tags_ext, seq_len[:, None], axis=1)[:, 0]
    gold = (
        np.where(mask, trans_sc + emit_sc, 0.0).sum(1, dtype=np.float64)
        + trans[STOP, last_tag]
    )
    return gold  # [B] f64


def kernel(feats, tags, seq_len, transitions):
    feats = np.asarray(feats)
    per_core, etil, ctx = _host_prep(feats, seq_len, transitions)
    nc = _build_module()
    in_maps = [{"etil": etil, **per_core[c]} for c in range(NCORES)]
    res = run_bass_kernel_spmd(nc, in_maps, list(range(NCORES)))
    outs = [np.asarray(res.results[c]["uout"]).astype(np.float64) for c in range(NCORES)]
    p1 = np.concatenate([o[:, 0:BC] for o in outs], axis=1)
    p2 = np.concatenate([o[:, BC:BC2] for o in outs], axis=1)
    p3 = np.concatenate([o[:, BC2 : BC2 + BC] for o in outs], axis=1)
    p4 = np.concatenate([o[:, BC2 + BC : 2 * BC2] for o in outs], axis=1)
    scores = _combine((p1, p2, p3, p4), ctx)
    gold = _gold_score(feats, tags, seq_len, transitions)
    loss = np.mean(scores - gold)
    return np.float32(loss)


# revision 7
# speedup vs baseline: 1.6771x; 1.1737x over previous
"""CRF negative log-likelihood loss on 8 Trainium2 NeuronCores.

Strategy: data-parallel over batch (64 sequences per core) with a 5-segment
split of each sequence cutting the serial scan depth to 205 device slots.
Segments (tau ranges): [1,205] [206,410] [411,615] [616,820] [821,1024].

  alpha: exact forward chain over seg1 (48 states + hold for L<=204)
  x_j = T_j . 1         (fwd-seeded through middle segment j = 2,3,4)
  y_j = T_j^T . e_stop  (bwd-seeded, also serves as the exact tail for
                         batches whose L falls inside segment j via a src
                         state injected at tau=L)
  b = A^T G_821 beta_821 (exact backward chain over seg5, src-injected)

Products of 205 positive transfer matrices are numerically rank-1
(Birkhoff contraction), so T_j w ~ x_j (y_j.w)/(y_j.1) and the partition
function composes from host-side dots:
  Z ~ (b.x4)(y4.x3)(y3.x2)(y2.alpha) / [(y2.1)(y3.1)(y4.1)]
truncated at the segment containing L.

Packing: 8 half-chains -> 4 packs of [98, 64] sharing one block [98,98]
bf16 stationary: P1=[alpha|y2] P2=[x2|y3] P3=[x3|y4] P4=[x4|b]. Packs are
PAIRED: (P1,P2) and (P3,P4) each write one [98,128] PSUM tile (two matmuls
into disjoint column slices) consumed by ONE wide DVE Hadamard, amortizing
the fixed PSUM-access cost; the two pairs are independent chains that
interleave on the engines to hide cross-engine sync latency. All chains
run in the exponential domain (bf16, fp32 PSUM) with host-precomputed
per-(batch,step) shifts; the gold path score is a host gather.
"""
import numpy as np
import ml_dtypes
from contextlib import ExitStack

import concourse.bacc as bacc
import concourse.bass as bass
import concourse.tile as tile
from concourse import mybir
from concourse.bass_utils import run_bass_kernel_spmd

B, T, K = 512, 1024, 48
START, STOP = 46, 47
NEG = -10000.0
HOLD = 48
SRCL = 48
KF = 49
K2 = 98
NCORES = 8
BC = B // NCORES   # 64
BC2 = 2 * BC       # 128 (pair width)
D = 205            # device slots
BND = [0, 205, 410, 615, 820, 1024]
CH = 41            # slots per emission chunk (205 = 5*41)

_nc_cache = {}


def _build_module(d_slots=D, ch=CH):
    key = ("nc", d_slots, ch)
    if key in _nc_cache:
        return _nc_cache[key]
    nc = bacc.Bacc(
        "TRN2",
        target_bir_lowering=False,
        debug=False,
        enable_asserts=False,
        num_devices=NCORES,
    )
    f32 = mybir.dt.float32
    bf16 = mybir.dt.bfloat16
    e_dram = nc.dram_tensor("etil", [K2, K2], bf16, kind="ExternalInput").ap()
    gA_dram = nc.dram_tensor("emis12", [K2, d_slots, BC2], bf16, kind="ExternalInput").ap()
    gB_dram = nc.dram_tensor("emis34", [K2, d_slots, BC2], bf16, kind="ExternalInput").ap()
    w0_dram = nc.dram_tensor("w0", [K2, 2 * BC2], bf16, kind="ExternalInput").ap()
    o_dram = nc.dram_tensor("uout", [K2, 2 * BC2], f32, kind="ExternalOutput").ap()

    with tile.TileContext(nc) as tc:
        with ExitStack() as ctx:
            const = ctx.enter_context(tc.tile_pool(name="const", bufs=1))
            wpool = ctx.enter_context(tc.tile_pool(name="wp", bufs=4))
            gexp_p = ctx.enter_context(tc.tile_pool(name="gexp", bufs=2))
            psum_p = ctx.enter_context(tc.tile_pool(name="ps", bufs=4, space="PSUM"))

            etile = const.tile([K2, K2], bf16)
            nc.sync.dma_start(out=etile, in_=e_dram)

            wA = const.tile([K2, BC2], bf16)
            nc.sync.dma_start(out=wA, in_=w0_dram[:, 0:BC2])
            wB = const.tile([K2, BC2], bf16)
            nc.sync.dma_start(out=wB, in_=w0_dram[:, BC2 : 2 * BC2])

            outA = const.tile([K2, BC2], f32)
            outB = const.tile([K2, BC2], f32)

            nstep = 0
            first = True
            while nstep < d_slots:
                ns = min(ch if not first else 9, d_slots - nstep)
                first = False
                geA = gexp_p.tile([K2, ch, BC2], bf16, tag="geA")
                nc.sync.dma_start(
                    out=geA[:, :ns, :], in_=gA_dram[:, nstep : nstep + ns, :]
                )
                geB = gexp_p.tile([K2, ch, BC2], bf16, tag="geB")
                nc.sync.dma_start(
                    out=geB[:, :ns, :], in_=gB_dram[:, nstep : nstep + ns, :]
                )
                for s in range(ns):
                    last = nstep + s == d_slots - 1
                    psA = psum_p.tile([K2, BC2], f32, tag="psA")
                    nc.tensor.matmul(psA, etile, wA, start=True, stop=True)
                    psB = psum_p.tile([K2, BC2], f32, tag="psB")
                    nc.tensor.matmul(psB, etile, wB, start=True, stop=True)
                    if last:
                        nc.vector.tensor_mul(outA, psA, geA[:, s, :])
                        nc.vector.tensor_mul(outB, psB, geB[:, s, :])
                    else:
                        wAn = wpool.tile([K2, BC2], bf16, tag="wA")
                        nc.vector.tensor_mul(wAn, psA, geA[:, s, :])
                        wA = wAn
                        wBn = wpool.tile([K2, BC2], bf16, tag="wB")
                        nc.vector.tensor_mul(wBn, psB, geB[:, s, :])
                        wB = wBn
                nstep += ns
            nc.sync.dma_start(out=o_dram[:, 0:BC2], in_=outA)
            nc.sync.dma_start(out=o_dram[:, BC2 : 2 * BC2], in_=outB)

    nc.compile()
    _nc_cache[key] = nc
    return nc


def _host_prep(feats, seq_len, trans):
    feats = np.ascontiguousarray(feats, dtype=np.float32)
    seq_len = np.asarray(seq_len, dtype=np.int64)
    trans = np.asarray(trans, dtype=np.float32)
    L = seq_len

    mx = feats.max(axis=2)
    E64 = np.exp(trans.astype(np.float64)).T

    drift = []
    for b in range(6):
        fv = np.full(K, NEG, dtype=np.float64)
        fv[START] = 0.0
        Lb = int(L[b])
        for t in range(min(Lb, 256)):
            m = fv.max()
            wv = np.exp(fv - m)
            with np.errstate(divide="ignore"):
                fv = np.log(E64.T @ wv) + m + feats[b, t]
            drift.append((fv.max() - m) - mx[b, t])
    mu = float(np.mean(drift))

    driftb = []
    nb = 0
    for b in range(B):
        Lb = int(L[b])
        if Lb < 700:
            continue
        nb += 1
        if nb > 6:
            break
        bv = trans[STOP, :].astype(np.float64).copy()
        for t in range(Lb, Lb - 200, -1):
            m = bv.max()
            wv = np.exp(bv - m)
            gv = np.exp(feats[b, t - 1].astype(np.float64))
            with np.errstate(divide="ignore"):
                bv = np.log(E64 @ (gv * wv)) + m
            driftb.append((bv.max() - m) - mx[b, t - 1])
    mub = float(np.mean(driftb)) if driftb else mu

    c = mx + mu
    cb = mx + mub
    Ccum = np.cumsum(c, axis=1, dtype=np.float64)
    Ccumb = np.cumsum(cb, axis=1, dtype=np.float64)
    estop = np.exp(trans[STOP, :K].astype(np.float64))
    estop32 = estop.astype(np.float32)

    n_ = np.arange(1, D + 1)
    gfall = np.exp(feats - c[:, :, None])
    gball = np.exp(feats - cb[:, :, None])

    # alpha rows [B, D, KF]
    ga = np.zeros((B, D, KF), dtype=np.float32)
    livef = n_[None, :] <= L[:, None]
    ga[:, :, :K] = np.where(livef[:, :, None], gfall[:, :D, :], 0.0)
    holdon = n_[None, :] >= (L[:, None] + 1)
    ga[:, :, HOLD] = np.where(holdon, 1.0, 0.0)

    def x_rows(j):
        bjm1, bj = BND[j - 1], BND[j]
        act = L > bj
        tau = bjm1 + n_
        g = np.where(act[:, None, None], gfall[:, tau - 1, :], 0.0).astype(np.float32)
        return g, act

    def y_rows(j):
        bjm1, bj = BND[j - 1], BND[j]
        act = L >= bjm1 + 1
        sstar = np.maximum(bj - L, 0)
        g = np.zeros((B, D, KF), dtype=np.float32)
        tau = bj - n_
        valid = (
            (n_[None, :] >= np.maximum(sstar, 1)[:, None])
            & (n_[None, :] <= D - 1)
            & act[:, None]
        )
        gv = gball[:, np.clip(tau - 1, 0, T - 1), :]
        g[:, :, :K] = np.where(valid[:, :, None], gv, 0.0)
        g[act, D - 1, :K] = 1.0
        srcon = (n_[None, :] < sstar[:, None]) & act[:, None]
        g[:, :, SRCL] = np.where(srcon, 1.0, 0.0)
        u0 = np.zeros((B, KF), dtype=np.float32)
        inj = act & (sstar == 0)
        u0[inj, :K] = gball[inj, bj - 1, :] * estop32[None, :]
        u0[act & (sstar > 0), SRCL] = 1.0
        return g, u0

    def b_rows():
        b4 = BND[4]
        act = L >= b4 + 1
        sstar = 1025 - L
        g = np.zeros((B, D, KF), dtype=np.float32)
        tau = 1025 - n_
        valid = (n_[None, :] >= sstar[:, None]) & (n_[None, :] <= D - 1) & act[:, None]
        gv = gball[:, np.clip(tau - 1, 0, T - 1), :]
        g[:, :, :K] = np.where(valid[:, :, None], gv, 0.0)
        g[act, D - 1, :K] = 1.0
        srcon = (n_[None, :] < sstar[:, None]) & act[:, None]
        g[:, :, SRCL] = np.where(srcon, 1.0, 0.0)
        u0 = np.zeros((B, KF), dtype=np.float32)
        u0[act, SRCL] = 1.0
        return g, u0

    gx2, actx2 = x_rows(2)
    gy2, u0y2 = y_rows(2)
    gx3, actx3 = x_rows(3)
    gy3, u0y3 = y_rows(3)
    gx4, actx4 = x_rows(4)
    gy4, u0y4 = y_rows(4)
    gb_, u0b = b_rows()

    def pack(gf, gbk):
        g = np.zeros((B, D, K2), dtype=np.float32)
        g[:, :, : gf.shape[2]] = gf
        g[:, :, KF : KF + gbk.shape[2]] = gbk
        return g

    P1 = pack(ga, gy2)
    P2 = pack(gx2, gy3)
    P3 = pack(gx3, gy4)
    P4 = pack(gx4, gb_)

    u1 = np.zeros((B, K2), dtype=np.float32)
    u1[:, START] = 1.0
    u1[:, KF:] = u0y2
    u2 = np.zeros((B, K2), dtype=np.float32)
    u2[actx2, :K] = 1.0
    u2[:, KF:] = u0y3
    u3 = np.zeros((B, K2), dtype=np.float32)
    u3[actx3, :K] = 1.0
    u3[:, KF:] = u0y4
    u4 = np.zeros((B, K2), dtype=np.float32)
    u4[actx4, :K] = 1.0
    u4[:, KF:] = u0b

    per_core = []
    for cix in range(NCORES):
        sl = slice(cix * BC, (cix + 1) * BC)
        # [K2, D, 128]: cols 0-63 = pack1, 64-127 = pack2 (same sequences)
        gA = np.concatenate(
            [P1[sl].transpose(2, 1, 0), P2[sl].transpose(2, 1, 0)], axis=2
        ).astype(ml_dtypes.bfloat16)
        gB = np.concatenate(
            [P3[sl].transpose(2, 1, 0), P4[sl].transpose(2, 1, 0)], axis=2
        ).astype(ml_dtypes.bfloat16)
        w0 = np.concatenate(
            [u1[sl].T, u2[sl].T, u3[sl].T, u4[sl].T], axis=1
        ).astype(ml_dtypes.bfloat16)
        per_core.append(
            {
                "emis12": np.ascontiguousarray(gA),
                "emis34": np.ascontiguousarray(gB),
                "w0": np.ascontiguousarray(w0),
            }
        )

    S_ = np.zeros((K2, K2), dtype=np.float32)
    S_[:K, :K] = np.exp(trans).T
    S_[:K, HOLD] = estop32
    S_[HOLD, HOLD] = 1.0
    S_[KF : KF + K, KF : KF + K] = np.exp(trans)
    S_[KF + SRCL, KF : KF + K] = estop32
    S_[KF + SRCL, KF + SRCL] = 1.0
    etil = S_.astype(ml_dtypes.bfloat16)

    ar = np.arange(B)
    ctx = {
        "seq_len": L,
        "estop": estop,
        "C_at_L": Ccum[ar, L - 1],
        "Cal": Ccum[:, BND[1] - 1],
        "Cx2": Ccum[:, BND[2] - 1] - Ccum[:, BND[1] - 1],
        "Cx3": Ccum[:, BND[3] - 1] - Ccum[:, BND[2] - 1],
        "Cx4": Ccum[:, BND[4] - 1] - Ccum[:, BND[3] - 1],
        "Cy2": Ccumb[ar, L - 1] - Ccumb[:, BND[1] - 1],
        "Cy3": Ccumb[ar, L - 1] - Ccumb[:, BND[2] - 1],
        "Cy4": Ccumb[ar, L - 1] - Ccumb[:, BND[3] - 1],
        "Cb": Ccumb[ar, L - 1] - Ccumb[:, BND[4] - 1],
    }
    return per_core, etil, ctx


def _combine(packs, ctx):
    """packs: (p1, p2, p3, p4) each [K2, B] f64; returns per-batch scores."""
    p1, p2, p3, p4 = packs
    L = ctx["seq_len"]
    estop = ctx["estop"]
    alpha = p1[:KF, :]
    y2 = p1[KF : KF + K, :]
    x2 = p2[:K, :]
    y3 = p2[KF : KF + K, :]
    x3 = p3[:K, :]
    y4 = p3[KF : KF + K, :]
    x4 = p4[:K, :]
    bv = p4[KF : KF + K, :]

    d_y2a = (y2 * alpha[:K, :]).sum(0)
    d_y3x2 = (y3 * x2).sum(0)
    d_y4x3 = (y4 * x3).sum(0)
    d_bx4 = (bv * x4).sum(0)
    n_y2 = y2.sum(0)
    n_y3 = y3.sum(0)
    n_y4 = y4.sum(0)

    scores = np.zeros(B)
    J1 = L <= BND[1] - 1
    scores[J1] = np.log(alpha[HOLD, J1]) + ctx["C_at_L"][J1]
    JM = L == BND[1]
    if JM.any():
        dm = (alpha[:K, :] * estop[:, None]).sum(0)
        scores[JM] = np.log(dm[JM]) + ctx["Cal"][JM]
    J2 = (L > BND[1]) & (L <= BND[2])
    scores[J2] = np.log(d_y2a[J2]) + ctx["Cal"][J2] + ctx["Cy2"][J2]
    J3 = (L > BND[2]) & (L <= BND[3])
    scores[J3] = (
        np.log(d_y3x2[J3]) + np.log(d_y2a[J3]) - np.log(n_y2[J3])
        + ctx["Cal"][J3] + ctx["Cx2"][J3] + ctx["Cy3"][J3]
    )
    J4 = (L > BND[3]) & (L <= BND[4])
    scores[J4] = (
        np.log(d_y4x3[J4]) + np.log(d_y3x2[J4]) + np.log(d_y2a[J4])
        - np.log(n_y2[J4]) - np.log(n_y3[J4])
        + ctx["Cal"][J4] + ctx["Cx2"][J4] + ctx["Cx3"][J4] + ctx["Cy4"][J4]
    )
    J5 = L > BND[4]
    scores[J5] = (
        np.log(d_bx4[J5]) + np.log(d_y4x3[J5]) + np.log(d_y3x2[J5])
        + np.log(d_y2a[J5]) - np.log(n_y2[J5]) - np.log(n_y3[J5])
        - np.log(n_y4[J5])
        + ctx["Cal"][J5] + ctx["Cx2"][J5] + ctx["Cx3"][J5] + ctx["Cx4"][J5]
        + ctx["Cb"][J5]
    )
    return scores


def _gold_score(feats, tags, seq_len, trans):
    feats = np.asarray(feats, dtype=np.float32)
    tags = np.asarray(tags, dtype=np.int64)
    seq_len = np.asarray(seq_len, dtype=np.int64)
    trans = np.asarray(trans, dtype=np.float32)
    tags_ext = np.concatenate(
        [np.full((B, 1), START, dtype=np.int64), tags], axis=1
    )
    trans_sc = trans[tags_ext[:, 1:], tags_ext[:, :-1]]
    emit_sc = np.take_along_axis(feats, tags_ext[:, 1:, None], axis=2)[..., 0]
    mask = np.arange(T)[None, :] < seq_len[:, None]
    last_tag = np.take_along_axis(tags_ext, seq_len[:, None], axis=1)[:, 0]
    gold = (
        np.where(mask, trans_sc + emit_sc, 0.0).sum(1, dtype=np.float64)
        + trans[STOP, last_tag]
    )
    return gold  # [B] f64


def kernel(feats, tags, seq_len, transitions):
    feats = np.asarray(feats)
    per_core, etil, ctx = _host_prep(feats, seq_len, transitions)
    nc = _build_module()
    in_maps = [{"etil": etil, **per_core[c]} for c in range(NCORES)]
    res = run_bass_kernel_spmd(nc, in_maps, list(range(NCORES)))
    outs = [np.asarray(res.results[c]["uout"]).astype(np.float64) for c in range(NCORES)]
    p1 = np.concatenate([o[:, 0:BC] for o in outs], axis=1)
    p2 = np.concatenate([o[:, BC:BC2] for o in outs], axis=1)
    p3 = np.concatenate([o[:, BC2 : BC2 + BC] for o in outs], axis=1)
    p4 = np.concatenate([o[:, BC2 + BC : 2 * BC2] for o in outs], axis=1)
    scores = _combine((p1, p2, p3, p4), ctx)
    gold = _gold_score(feats, tags, seq_len, transitions)
    loss = np.mean(scores - gold)
    return np.float32(loss)


# revision 8
# speedup vs baseline: 1.9435x; 1.1588x over previous
"""CRF negative log-likelihood loss on 8 Trainium2 NeuronCores.

Strategy: data-parallel over batch (64 sequences per core) with an
M=9-segment split of each sequence cutting the serial scan depth to 114
device slots. Boundaries BND[j] = j*114 (last segment 912..1024).

  alpha: exact forward chain over seg1 (48 states + hold for short L)
  x_j = T_j . 1         (fwd-seeded through middle segment j)
  y_j = T_j^T . e_stop  (bwd-seeded; doubles as the exact tail for batches
                         whose L falls inside segment j via a src state
                         injected at tau=L)
  b = A^T G_{913} beta_{913} (exact backward chain over the last segment)

Products of >=114 positive transfer matrices are numerically rank-1
(Birkhoff contraction), so T_j w ~ x_j (y_j.w)/(y_j.1) and the partition
function composes from host-side dots telescoping across segments:
  Z ~ (b.x_{M-1}) prod_j [(y_j.x_{j-1})/(y_j.1)] (y_2.alpha)
truncated at the segment containing L.

Packing: the 2(M-1) half-chains form M-1=8 packs sharing one block [98,98]
bf16 stationary: P1=[alpha|y2], Pj=[x_j|y_{j+1}], P8=[x_8|b]. Packs are
grouped into TWO super-chains of 4 packs, each a [98, 256] datapath: one
PE matmul + one wide DVE Hadamard per slot (the wide TT amortizes the
fixed 125ns PSUM-access cost over 256 columns). The two super-chains are
independent and interleave on the engines, hiding cross-engine sync
latency. All chains run in the exponential domain (bf16, fp32 PSUM) with
host-precomputed per-(batch,step) shifts; the gold path score is a cheap
host gather.
"""
import numpy as np
import ml_dtypes
from contextlib import ExitStack

import concourse.bacc as bacc
import concourse.bass as bass
import concourse.tile as tile
from concourse import mybir
from concourse.bass_utils import run_bass_kernel_spmd

B, T, K = 512, 1024, 48
START, STOP = 46, 47
NEG = -10000.0
HOLD = 48
SRCL = 48
KF = 49
K2 = 98
NCORES = 8
BC = B // NCORES    # 64
M = 9               # segments
NP = M - 1          # packs (8)
NG = NP // 2        # packs per super-chain (4)
W = NG * BC         # super-chain width (256)
D = -(-1025 // M)   # 114 device slots
BND = [j * D for j in range(M)] + [1024]
CH = 38             # slots per emission chunk (114 = 3*38)

_nc_cache = {}


def _build_module(d_slots=D, ch=CH):
    key = ("nc", d_slots, ch)
    if key in _nc_cache:
        return _nc_cache[key]
    nc = bacc.Bacc(
        "TRN2",
        target_bir_lowering=False,
        debug=False,
        enable_asserts=False,
        num_devices=NCORES,
    )
    f32 = mybir.dt.float32
    bf16 = mybir.dt.bfloat16
    e_dram = nc.dram_tensor("etil", [K2, K2], bf16, kind="ExternalInput").ap()
    gA_dram = nc.dram_tensor("emisA", [K2, d_slots, W], bf16, kind="ExternalInput").ap()
    gB_dram = nc.dram_tensor("emisB", [K2, d_slots, W], bf16, kind="ExternalInput").ap()
    w0_dram = nc.dram_tensor("w0", [K2, 2 * W], bf16, kind="ExternalInput").ap()
    o_dram = nc.dram_tensor("uout", [K2, 2 * W], f32, kind="ExternalOutput").ap()

    with tile.TileContext(nc) as tc:
        with ExitStack() as ctx:
            const = ctx.enter_context(tc.tile_pool(name="const", bufs=1))
            wpool = ctx.enter_context(tc.tile_pool(name="wp", bufs=4))
            gexp_p = ctx.enter_context(tc.tile_pool(name="gexp", bufs=2))
            psum_p = ctx.enter_context(tc.tile_pool(name="ps", bufs=4, space="PSUM"))

            etile = const.tile([K2, K2], bf16)
            nc.sync.dma_start(out=etile, in_=e_dram)

            wA = const.tile([K2, W], bf16)
            nc.sync.dma_start(out=wA, in_=w0_dram[:, 0:W])
            wB = const.tile([K2, W], bf16)
            nc.sync.dma_start(out=wB, in_=w0_dram[:, W : 2 * W])

            outA = const.tile([K2, W], f32)
            outB = const.tile([K2, W], f32)

            nstep = 0
            first = True
            while nstep < d_slots:
                ns = min(ch if not first else 9, d_slots - nstep)
                first = False
                geA = gexp_p.tile([K2, ch, W], bf16, tag="geA")
                nc.sync.dma_start(
                    out=geA[:, :ns, :], in_=gA_dram[:, nstep : nstep + ns, :]
                )
                geB = gexp_p.tile([K2, ch, W], bf16, tag="geB")
                nc.sync.dma_start(
                    out=geB[:, :ns, :], in_=gB_dram[:, nstep : nstep + ns, :]
                )
                for s in range(ns):
                    last = nstep + s == d_slots - 1
                    psA = psum_p.tile([K2, W], f32, tag="psA")
                    nc.tensor.matmul(psA, etile, wA, start=True, stop=True)
                    psB = psum_p.tile([K2, W], f32, tag="psB")
                    nc.tensor.matmul(psB, etile, wB, start=True, stop=True)
                    if last:
                        nc.vector.tensor_mul(outA, psA, geA[:, s, :])
                        nc.vector.tensor_mul(outB, psB, geB[:, s, :])
                    else:
                        wAn = wpool.tile([K2, W], bf16, tag="wA")
                        nc.vector.tensor_mul(wAn, psA, geA[:, s, :])
                        wA = wAn
                        wBn = wpool.tile([K2, W], bf16, tag="wB")
                        nc.vector.tensor_mul(wBn, psB, geB[:, s, :])
                        wB = wBn
                nstep += ns
            nc.sync.dma_start(out=o_dram[:, 0:W], in_=outA)
            nc.sync.dma_start(out=o_dram[:, W : 2 * W], in_=outB)

    nc.compile()
    _nc_cache[key] = nc
    return nc


def _host_prep(feats, seq_len, trans):
    feats = np.ascontiguousarray(feats, dtype=np.float32)
    seq_len = np.asarray(seq_len, dtype=np.int64)
    trans = np.asarray(trans, dtype=np.float32)
    L = seq_len

    mx = feats.max(axis=2)
    E64 = np.exp(trans.astype(np.float64)).T

    drift = []
    for b in range(6):
        fv = np.full(K, NEG, dtype=np.float64)
        fv[START] = 0.0
        Lb = int(L[b])
        for t in range(min(Lb, 256)):
            m = fv.max()
            wv = np.exp(fv - m)
            with np.errstate(divide="ignore"):
                fv = np.log(E64.T @ wv) + m + feats[b, t]
            drift.append((fv.max() - m) - mx[b, t])
    mu = float(np.mean(drift))

    driftb = []
    nb = 0
    for b in range(B):
        Lb = int(L[b])
        if Lb < 700:
            continue
        nb += 1
        if nb > 6:
            break
        bv = trans[STOP, :].astype(np.float64).copy()
        for t in range(Lb, Lb - 200, -1):
            m = bv.max()
            wv = np.exp(bv - m)
            gv = np.exp(feats[b, t - 1].astype(np.float64))
            with np.errstate(divide="ignore"):
                bv = np.log(E64 @ (gv * wv)) + m
            driftb.append((bv.max() - m) - mx[b, t - 1])
    mub = float(np.mean(driftb)) if driftb else mu

    c = mx + mu
    cb = mx + mub
    Ccum = np.cumsum(c, axis=1, dtype=np.float64)
    Ccumb = np.cumsum(cb, axis=1, dtype=np.float64)
    estop = np.exp(trans[STOP, :K].astype(np.float64))
    estop32 = estop.astype(np.float32)

    n_ = np.arange(1, D + 1)
    gfall = np.exp(feats - c[:, :, None])
    gball = np.exp(feats - cb[:, :, None])

    ga = np.zeros((B, D, KF), dtype=np.float32)
    livef = n_[None, :] <= L[:, None]
    ga[:, :, :K] = np.where(livef[:, :, None], gfall[:, :D, :], 0.0)
    holdon = n_[None, :] >= (L[:, None] + 1)
    ga[:, :, HOLD] = np.where(holdon, 1.0, 0.0)

    def x_rows(j):
        bjm1, bj = BND[j - 1], BND[j]
        act = L > bj
        tau = bjm1 + n_
        g = np.where(
            act[:, None, None], gfall[:, np.clip(tau - 1, 0, T - 1), :], 0.0
        ).astype(np.float32)
        return g, act

    def y_rows(j):
        bjm1, bj = BND[j - 1], BND[j]
        act = L >= bjm1 + 1
        sstar = np.maximum(bj - L, 0)
        g = np.zeros((B, D, KF), dtype=np.float32)
        tau = bj - n_
        valid = (
            (n_[None, :] >= np.maximum(sstar, 1)[:, None])
            & (n_[None, :] <= D - 1)
            & act[:, None]
        )
        gv = gball[:, np.clip(tau - 1, 0, T - 1), :]
        g[:, :, :K] = np.where(valid[:, :, None], gv, 0.0)
        g[act, D - 1, :K] = 1.0
        srcon = (n_[None, :] < sstar[:, None]) & act[:, None]
        g[:, :, SRCL] = np.where(srcon, 1.0, 0.0)
        u0 = np.zeros((B, KF), dtype=np.float32)
        inj = act & (sstar == 0)
        u0[inj, :K] = gball[inj, bj - 1, :] * estop32[None, :]
        u0[act & (sstar > 0), SRCL] = 1.0
        return g, u0

    def b_rows():
        bm1 = BND[M - 1]
        act = L >= bm1 + 1
        sstar = bm1 + 1 + D - L
        g = np.zeros((B, D, KF), dtype=np.float32)
        tau = bm1 + 1 + D - n_
        valid = (
            (n_[None, :] >= sstar[:, None]) & (n_[None, :] <= D - 1) & act[:, None]
        )
        gv = gball[:, np.clip(tau - 1, 0, T - 1), :]
        g[:, :, :K] = np.where(valid[:, :, None], gv, 0.0)
        g[act, D - 1, :K] = 1.0
        srcon = (n_[None, :] < sstar[:, None]) & act[:, None]
        g[:, :, SRCL] = np.where(srcon, 1.0, 0.0)
        u0 = np.zeros((B, KF), dtype=np.float32)
        u0[act, SRCL] = 1.0
        return g, u0

    xs = {}
    ys = {}
    acts = {}
    for j in range(2, M):
        xs[j], acts[j] = x_rows(j)
        ys[j] = y_rows(j)
    gb_, u0b = b_rows()

    def pack(gf, gbk):
        g = np.zeros((B, D, K2), dtype=np.float32)
        g[:, :, : gf.shape[2]] = gf
        g[:, :, KF : KF + gbk.shape[2]] = gbk
        return g

    P = [pack(ga, ys[2][0])]
    U = []
    u = np.zeros((B, K2), dtype=np.float32)
    u[:, START] = 1.0
    u[:, KF:] = ys[2][1]
    U.append(u)
    for j in range(2, M - 1):
        P.append(pack(xs[j], ys[j + 1][0]))
        u = np.zeros((B, K2), dtype=np.float32)
        u[acts[j], :K] = 1.0
        u[:, KF:] = ys[j + 1][1]
        U.append(u)
    P.append(pack(xs[M - 1], gb_))
    u = np.zeros((B, K2), dtype=np.float32)
    u[acts[M - 1], :K] = 1.0
    u[:, KF:] = u0b
    U.append(u)

    per_core = []
    for cix in range(NCORES):
        sl = slice(cix * BC, (cix + 1) * BC)
        gA = np.concatenate(
            [P[k][sl].transpose(2, 1, 0) for k in range(NG)], axis=2
        ).astype(ml_dtypes.bfloat16)
        gB = np.concatenate(
            [P[k][sl].transpose(2, 1, 0) for k in range(NG, NP)], axis=2
        ).astype(ml_dtypes.bfloat16)
        w0 = np.concatenate([U[k][sl].T for k in range(NP)], axis=1).astype(
            ml_dtypes.bfloat16
        )
        per_core.append(
            {
                "emisA": np.ascontiguousarray(gA),
                "emisB": np.ascontiguousarray(gB),
                "w0": np.ascontiguousarray(w0),
            }
        )

    S_ = np.zeros((K2, K2), dtype=np.float32)
    S_[:K, :K] = np.exp(trans).T
    S_[:K, HOLD] = estop32
    S_[HOLD, HOLD] = 1.0
    S_[KF : KF + K, KF : KF + K] = np.exp(trans)
    S_[KF + SRCL, KF : KF + K] = estop32
    S_[KF + SRCL, KF + SRCL] = 1.0
    etil = S_.astype(ml_dtypes.bfloat16)

    ar = np.arange(B)
    ctx = {
        "seq_len": L,
        "estop": estop,
        "C_at_L": Ccum[ar, L - 1],
        "Cal": Ccum[:, BND[1] - 1],
        "Cx": {j: Ccum[:, BND[j] - 1] - Ccum[:, BND[j - 1] - 1] for j in range(2, M)},
        "CyL": {j: Ccumb[ar, L - 1] - Ccumb[:, BND[j - 1] - 1] for j in range(2, M)},
        "Cb": Ccumb[ar, L - 1] - Ccumb[:, BND[M - 1] - 1],
    }
    return per_core, etil, ctx


def _combine(packs, ctx):
    """packs: list of NP arrays [K2, B] f64; returns per-batch scores."""
    L = ctx["seq_len"]
    estop = ctx["estop"]
    alpha = packs[0][:KF, :]
    xv = {}
    yv = {2: packs[0][KF : KF + K, :]}
    for j in range(2, M):
        xv[j] = packs[j - 1][:K, :]
    for j in range(2, M - 1):
        yv[j + 1] = packs[j - 1][KF : KF + K, :]
    bv = packs[NP - 1][KF : KF + K, :]

    d = {2: (yv[2] * alpha[:K, :]).sum(0)}
    n = {2: yv[2].sum(0)}
    for j in range(3, M):
        d[j] = (yv[j] * xv[j - 1]).sum(0)
        n[j] = yv[j].sum(0)
    d_b = (bv * xv[M - 1]).sum(0)

    scores = np.zeros(B)
    J1 = L <= BND[1] - 1
    scores[J1] = np.log(alpha[HOLD, J1]) + ctx["C_at_L"][J1]
    JM_ = L == BND[1]
    if JM_.any():
        dm = (alpha[:K, :] * estop[:, None]).sum(0)
        scores[JM_] = np.log(dm[JM_]) + ctx["Cal"][JM_]
    with np.errstate(divide="ignore", invalid="ignore"):
        for J in range(2, M):
            msk = (L > BND[J - 1]) & (L <= BND[J])
            sc = np.log(d[J]) + ctx["Cal"] + ctx["CyL"][J]
            for j in range(2, J):
                sc = sc + np.log(d[j]) - np.log(n[j]) + ctx["Cx"][j]
            scores[msk] = sc[msk]
        mskM = L > BND[M - 1]
        sc = np.log(d_b) + ctx["Cal"] + ctx["Cb"]
        for j in range(2, M):
            sc = sc + np.log(d[j]) - np.log(n[j]) + ctx["Cx"][j]
        scores[mskM] = sc[mskM]
    return scores


def _gold_score(feats, tags, seq_len, trans):
    feats = np.asarray(feats, dtype=np.float32)
    tags = np.asarray(tags, dtype=np.int64)
    seq_len = np.asarray(seq_len, dtype=np.int64)
    trans = np.asarray(trans, dtype=np.float32)
    tags_ext = np.concatenate(
        [np.full((B, 1), START, dtype=np.int64), tags], axis=1
    )
    trans_sc = trans[tags_ext[:, 1:], tags_ext[:, :-1]]
    emit_sc = np.take_along_axis(feats, tags_ext[:, 1:, None], axis=2)[..., 0]
    mask = np.arange(T)[None, :] < seq_len[:, None]
    last_tag = np.take_along_axis(tags_ext, seq_len[:, None], axis=1)[:, 0]
    gold = (
        np.where(mask, trans_sc + emit_sc, 0.0).sum(1, dtype=np.float64)
        + trans[STOP, last_tag]
    )
    return gold  # [B] f64


def kernel(feats, tags, seq_len, transitions):
    feats = np.asarray(feats)
    per_core, etil, ctx = _host_prep(feats, seq_len, transitions)
    nc = _build_module()
    in_maps = [{"etil": etil, **per_core[c]} for c in range(NCORES)]
    res = run_bass_kernel_spmd(nc, in_maps, list(range(NCORES)))
    outs = [np.asarray(res.results[c]["uout"]).astype(np.float64) for c in range(NCORES)]
    packs = [
        np.concatenate([o[:, k * BC : (k + 1) * BC] for o in outs], axis=1)
        for k in range(NP)
    ]
    scores = _combine(packs, ctx)
    gold = _gold_score(feats, tags, seq_len, transitions)
    loss = np.mean(scores - gold)
    return np.float32(loss)


# revision 9
# speedup vs baseline: 2.0994x; 1.0802x over previous
"""CRF negative log-likelihood loss on 8 Trainium2 NeuronCores.

Strategy: data-parallel over batch (64 sequences per core) with an
M=9-segment split of each sequence cutting the serial scan depth to 114
device slots. Boundaries BND[j] = j*114 (last segment 912..1024).

  alpha: exact forward chain over seg1 (48 states + hold for short L)
  x_j = T_j . 1         (fwd-seeded through middle segment j)
  y_j = T_j^T . e_stop  (bwd-seeded; doubles as the exact tail for batches
                         whose L falls inside segment j via a src state
                         injected at tau=L)
  b = A^T G_{913} beta_{913} (exact backward chain over the last segment)

Products of >=114 positive transfer matrices are numerically rank-1
(Birkhoff contraction), so T_j w ~ x_j (y_j.w)/(y_j.1) and the partition
function composes from host-side dots telescoping across segments:
  Z ~ (b.x_{M-1}) prod_j [(y_j.x_{j-1})/(y_j.1)] (y_2.alpha)
truncated at the segment containing L.

Packing: the 2(M-1) half-chains form M-1=8 packs sharing one block [98,98]
bf16 stationary: P1=[alpha|y2], Pj=[x_j|y_{j+1}], P8=[x_8|b]. Packs are
grouped into TWO super-chains of 4 packs, each a [98, 256] datapath: one
PE matmul + one wide DVE Hadamard per slot (the wide TT amortizes the
fixed 125ns PSUM-access cost over 256 columns). The two super-chains are
independent and interleave on the engines, hiding cross-engine sync
latency. All chains run in the exponential domain (bf16, fp32 PSUM) with
host-precomputed per-(batch,step) shifts; the gold path score is a cheap
host gather.
"""
import numpy as np
import ml_dtypes
from contextlib import ExitStack

import concourse.bacc as bacc
import concourse.bass as bass
import concourse.tile as tile
from concourse import mybir
from concourse.bass_utils import run_bass_kernel_spmd

B, T, K = 512, 1024, 48
START, STOP = 46, 47
NEG = -10000.0
HOLD = 48
SRCL = 48
KF = 49
K2 = 98
NCORES = 8
BC = B // NCORES    # 64
M = 9               # segments
NP = M - 1          # packs (8)
NG = NP // 2        # packs per super-chain (4)
W = NG * BC         # super-chain width (256)
D = -(-1025 // M)   # 114 device slots
BND = [j * D for j in range(M)] + [1024]
CH = 38             # slots per emission chunk (114 = 3*38)

_nc_cache = {}


def _build_module(d_slots=D, ch=CH):
    key = ("nc", d_slots, ch)
    if key in _nc_cache:
        return _nc_cache[key]
    nc = bacc.Bacc(
        "TRN2",
        target_bir_lowering=False,
        debug=False,
        enable_asserts=False,
        num_devices=NCORES,
    )
    f32 = mybir.dt.float32
    bf16 = mybir.dt.bfloat16
    e_dram = nc.dram_tensor("etil", [K2, K2], bf16, kind="ExternalInput").ap()
    gA_dram = nc.dram_tensor("emisA", [K2, d_slots, W], bf16, kind="ExternalInput").ap()
    gB_dram = nc.dram_tensor("emisB", [K2, d_slots, W], bf16, kind="ExternalInput").ap()
    w0_dram = nc.dram_tensor("w0", [K2, 2 * W], bf16, kind="ExternalInput").ap()
    o_dram = nc.dram_tensor("uout", [K2, 2 * W], f32, kind="ExternalOutput").ap()

    with tile.TileContext(nc) as tc:
        with ExitStack() as ctx:
            const = ctx.enter_context(tc.tile_pool(name="const", bufs=1))
            wpool = ctx.enter_context(tc.tile_pool(name="wp", bufs=4))
            gexp_p = ctx.enter_context(tc.tile_pool(name="gexp", bufs=3))
            psum_p = ctx.enter_context(tc.tile_pool(name="ps", bufs=4, space="PSUM"))

            etile = const.tile([K2, K2], bf16)
            nc.sync.dma_start(out=etile, in_=e_dram)

            wA = const.tile([K2, W], bf16)
            nc.sync.dma_start(out=wA, in_=w0_dram[:, 0:W])
            wB = const.tile([K2, W], bf16)
            nc.sync.dma_start(out=wB, in_=w0_dram[:, W : 2 * W])

            outA = const.tile([K2, W], f32)
            outB = const.tile([K2, W], f32)

            nstep = 0
            sched = [9, 12, 17] + [ch] * 100
            while nstep < d_slots:
                ns = min(sched.pop(0), d_slots - nstep)
                geA = gexp_p.tile([K2, ch, W], bf16, tag="geA")
                nc.sync.dma_start(
                    out=geA[:, :ns, :], in_=gA_dram[:, nstep : nstep + ns, :]
                )
                geB = gexp_p.tile([K2, ch, W], bf16, tag="geB")
                nc.sync.dma_start(
                    out=geB[:, :ns, :], in_=gB_dram[:, nstep : nstep + ns, :]
                )
                for s in range(ns):
                    last = nstep + s == d_slots - 1
                    psA = psum_p.tile([K2, W], f32, tag="psA")
                    nc.tensor.matmul(psA, etile, wA, start=True, stop=True)
                    psB = psum_p.tile([K2, W], f32, tag="psB")
                    nc.tensor.matmul(psB, etile, wB, start=True, stop=True)
                    if last:
                        nc.vector.tensor_mul(outA, psA, geA[:, s, :])
                        nc.vector.tensor_mul(outB, psB, geB[:, s, :])
                    else:
                        wAn = wpool.tile([K2, W], bf16, tag="wA")
                        nc.vector.tensor_mul(wAn, psA, geA[:, s, :])
                        wA = wAn
                        wBn = wpool.tile([K2, W], bf16, tag="wB")
                        nc.vector.tensor_mul(wBn, psB, geB[:, s, :])
                        wB = wBn
                nstep += ns
            nc.sync.dma_start(out=o_dram[:, 0:W], in_=outA)
            nc.sync.dma_start(out=o_dram[:, W : 2 * W], in_=outB)

    nc.compile()
    _nc_cache[key] = nc
    return nc


def _host_prep(feats, seq_len, trans):
    feats = np.ascontiguousarray(feats, dtype=np.float32)
    seq_len = np.asarray(seq_len, dtype=np.int64)
    trans = np.asarray(trans, dtype=np.float32)
    L = seq_len

    mx = feats.max(axis=2)
    E64 = np.exp(trans.astype(np.float64)).T

    drift = []
    for b in range(6):
        fv = np.full(K, NEG, dtype=np.float64)
        fv[START] = 0.0
        Lb = int(L[b])
        for t in range(min(Lb, 256)):
            m = fv.max()
            wv = np.exp(fv - m)
            with np.errstate(divide="ignore"):
                fv = np.log(E64.T @ wv) + m + feats[b, t]
            drift.append((fv.max() - m) - mx[b, t])
    mu = float(np.mean(drift))

    driftb = []
    nb = 0
    for b in range(B):
        Lb = int(L[b])
        if Lb < 700:
            continue
        nb += 1
        if nb > 6:
            break
        bv = trans[STOP, :].astype(np.float64).copy()
        for t in range(Lb, Lb - 200, -1):
            m = bv.max()
            wv = np.exp(bv - m)
            gv = np.exp(feats[b, t - 1].astype(np.float64))
            with np.errstate(divide="ignore"):
                bv = np.log(E64 @ (gv * wv)) + m
            driftb.append((bv.max() - m) - mx[b, t - 1])
    mub = float(np.mean(driftb)) if driftb else mu

    c = mx + mu
    cb = mx + mub
    Ccum = np.cumsum(c, axis=1, dtype=np.float64)
    Ccumb = np.cumsum(cb, axis=1, dtype=np.float64)
    estop = np.exp(trans[STOP, :K].astype(np.float64))
    estop32 = estop.astype(np.float32)

    n_ = np.arange(1, D + 1)
    gfall = np.exp(feats - c[:, :, None])
    gball = np.exp(feats - cb[:, :, None])

    ga = np.zeros((B, D, KF), dtype=np.float32)
    livef = n_[None, :] <= L[:, None]
    ga[:, :, :K] = np.where(livef[:, :, None], gfall[:, :D, :], 0.0)
    holdon = n_[None, :] >= (L[:, None] + 1)
    ga[:, :, HOLD] = np.where(holdon, 1.0, 0.0)

    def x_rows(j):
        bjm1, bj = BND[j - 1], BND[j]
        act = L > bj
        tau = bjm1 + n_
        g = np.where(
            act[:, None, None], gfall[:, np.clip(tau - 1, 0, T - 1), :], 0.0
        ).astype(np.float32)
        return g, act

    def y_rows(j):
        bjm1, bj = BND[j - 1], BND[j]
        act = L >= bjm1 + 1
        sstar = np.maximum(bj - L, 0)
        g = np.zeros((B, D, KF), dtype=np.float32)
        tau = bj - n_
        valid = (
            (n_[None, :] >= np.maximum(sstar, 1)[:, None])
            & (n_[None, :] <= D - 1)
            & act[:, None]
        )
        gv = gball[:, np.clip(tau - 1, 0, T - 1), :]
        g[:, :, :K] = np.where(valid[:, :, None], gv, 0.0)
        g[act, D - 1, :K] = 1.0
        srcon = (n_[None, :] < sstar[:, None]) & act[:, None]
        g[:, :, SRCL] = np.where(srcon, 1.0, 0.0)
        u0 = np.zeros((B, KF), dtype=np.float32)
        inj = act & (sstar == 0)
        u0[inj, :K] = gball[inj, bj - 1, :] * estop32[None, :]
        u0[act & (sstar > 0), SRCL] = 1.0
        return g, u0

    def b_rows():
        bm1 = BND[M - 1]
        act = L >= bm1 + 1
        sstar = bm1 + 1 + D - L
        g = np.zeros((B, D, KF), dtype=np.float32)
        tau = bm1 + 1 + D - n_
        valid = (
            (n_[None, :] >= sstar[:, None]) & (n_[None, :] <= D - 1) & act[:, None]
        )
        gv = gball[:, np.clip(tau - 1, 0, T - 1), :]
        g[:, :, :K] = np.where(valid[:, :, None], gv, 0.0)
        g[act, D - 1, :K] = 1.0
        srcon = (n_[None, :] < sstar[:, None]) & act[:, None]
        g[:, :, SRCL] = np.where(srcon, 1.0, 0.0)
        u0 = np.zeros((B, KF), dtype=np.float32)
        u0[act, SRCL] = 1.0
        return g, u0

    xs = {}
    ys = {}
    acts = {}
    for j in range(2, M):
        xs[j], acts[j] = x_rows(j)
        ys[j] = y_rows(j)
    gb_, u0b = b_rows()

    def pack(gf, gbk):
        g = np.zeros((B, D, K2), dtype=np.float32)
        g[:, :, : gf.shape[2]] = gf
        g[:, :, KF : KF + gbk.shape[2]] = gbk
        return g

    P = [pack(ga, ys[2][0])]
    U = []
    u = np.zeros((B, K2), dtype=np.float32)
    u[:, START] = 1.0
    u[:, KF:] = ys[2][1]
    U.append(u)
    for j in range(2, M - 1):
        P.append(pack(xs[j], ys[j + 1][0]))
        u = np.zeros((B, K2), dtype=np.float32)
        u[acts[j], :K] = 1.0
        u[:, KF:] = ys[j + 1][1]
        U.append(u)
    P.append(pack(xs[M - 1], gb_))
    u = np.zeros((B, K2), dtype=np.float32)
    u[acts[M - 1], :K] = 1.0
    u[:, KF:] = u0b
    U.append(u)

    per_core = []
    for cix in range(NCORES):
        sl = slice(cix * BC, (cix + 1) * BC)
        gA = np.concatenate(
            [P[k][sl].transpose(2, 1, 0) for k in range(NG)], axis=2
        ).astype(ml_dtypes.bfloat16)
        gB = np.concatenate(
            [P[k][sl].transpose(2, 1, 0) for k in range(NG, NP)], axis=2
        ).astype(ml_dtypes.bfloat16)
        w0 = np.concatenate([U[k][sl].T for k in range(NP)], axis=1).astype(
            ml_dtypes.bfloat16
        )
        per_core.append(
            {
                "emisA": np.ascontiguousarray(gA),
                "emisB": np.ascontiguousarray(gB),
                "w0": np.ascontiguousarray(w0),
            }
        )

    S_ = np.zeros((K2, K2), dtype=np.float32)
    S_[:K, :K] = np.exp(trans).T
    S_[:K, HOLD] = estop32
    S_[HOLD, HOLD] = 1.0
    S_[KF : KF + K, KF : KF + K] = np.exp(trans)
    S_[KF + SRCL, KF : KF + K] = estop32
    S_[KF + SRCL, KF + SRCL] = 1.0
    etil = S_.astype(ml_dtypes.bfloat16)

    ar = np.arange(B)
    ctx = {
        "seq_len": L,
        "estop": estop,
        "C_at_L": Ccum[ar, L - 1],
        "Cal": Ccum[:, BND[1] - 1],
        "Cx": {j: Ccum[:, BND[j] - 1] - Ccum[:, BND[j - 1] - 1] for j in range(2, M)},
        "CyL": {j: Ccumb[ar, L - 1] - Ccumb[:, BND[j - 1] - 1] for j in range(2, M)},
        "Cb": Ccumb[ar, L - 1] - Ccumb[:, BND[M - 1] - 1],
    }
    return per_core, etil, ctx


def _combine(packs, ctx):
    """packs: list of NP arrays [K2, B] f64; returns per-batch scores."""
    L = ctx["seq_len"]
    estop = ctx["estop"]
    alpha = packs[0][:KF, :]
    xv = {}
    yv = {2: packs[0][KF : KF + K, :]}
    for j in range(2, M):
        xv[j] = packs[j - 1][:K, :]
    for j in range(2, M - 1):
        yv[j + 1] = packs[j - 1][KF : KF + K, :]
    bv = packs[NP - 1][KF : KF + K, :]

    d = {2: (yv[2] * alpha[:K, :]).sum(0)}
    n = {2: yv[2].sum(0)}
    for j in range(3, M):
        d[j] = (yv[j] * xv[j - 1]).sum(0)
        n[j] = yv[j].sum(0)
    d_b = (bv * xv[M - 1]).sum(0)

    scores = np.zeros(B)
    J1 = L <= BND[1] - 1
    scores[J1] = np.log(alpha[HOLD, J1]) + ctx["C_at_L"][J1]
    JM_ = L == BND[1]
    if JM_.any():
        dm = (alpha[:K, :] * estop[:, None]).sum(0)
        scores[JM_] = np.log(dm[JM_]) + ctx["Cal"][JM_]
    with np.errstate(divide="ignore", invalid="ignore"):
        for J in range(2, M):
            msk = (L > BND[J - 1]) & (L <= BND[J])
            sc = np.log(d[J]) + ctx["Cal"] + ctx["CyL"][J]
            for j in range(2, J):
                sc = sc + np.log(d[j]) - np.log(n[j]) + ctx["Cx"][j]
            scores[msk] = sc[msk]
        mskM = L > BND[M - 1]
        sc = np.log(d_b) + ctx["Cal"] + ctx["Cb"]
        for j in range(2, M):
            sc = sc + np.log(d[j]) - np.log(n[j]) + ctx["Cx"][j]
        scores[mskM] = sc[mskM]
    return scores


def _gold_score(feats, tags, seq_len, trans):
    feats = np.asarray(feats, dtype=np.float32)
    tags = np.asarray(tags, dtype=np.int64)
    seq_len = np.asarray(seq_len, dtype=np.int64)
    trans = np.asarray(trans, dtype=np.float32)
    tags_ext = np.concatenate(
        [np.full((B, 1), START, dtype=np.int64), tags], axis=1
    )
    trans_sc = trans[tags_ext[:, 1:], tags_ext[:, :-1]]
    emit_sc = np.take_along_axis(feats, tags_ext[:, 1:, None], axis=2)[..., 0]
    mask = np.arange(T)[None, :] < seq_len[:, None]
    last_tag = np.take_along_axis(tags_ext, seq_len[:, None], axis=1)[:, 0]
    gold = (
        np.where(mask, trans_sc + emit_sc, 0.0).sum(1, dtype=np.float64)
        + trans[STOP, last_tag]
    )
    return gold  # [B] f64


def kernel(feats, tags, seq_len, transitions):
    feats = np.asarray(feats)
    per_core, etil, ctx = _host_prep(feats, seq_len, transitions)
    nc = _build_module()
    in_maps = [{"etil": etil, **per_core[c]} for c in range(NCORES)]
    res = run_bass_kernel_spmd(nc, in_maps, list(range(NCORES)))
    outs = [np.asarray(res.results[c]["uout"]).astype(np.float64) for c in range(NCORES)]
    packs = [
        np.concatenate([o[:, k * BC : (k + 1) * BC] for o in outs], axis=1)
        for k in range(NP)
    ]
    scores = _combine(packs, ctx)
    gold = _gold_score(feats, tags, seq_len, transitions)
    loss = np.mean(scores - gold)
    return np.float32(loss)


# revision 10
# speedup vs baseline: 2.1106x; 1.0054x over previous
"""CRF negative log-likelihood loss on 8 Trainium2 NeuronCores.

Strategy: data-parallel over batch (64 sequences per core) with an
M=9-segment split of each sequence cutting the serial scan depth to 114
device slots. Boundaries BND[j] = j*114 (last segment 912..1024).

  alpha: exact forward chain over seg1 (48 states + hold for short L)
  x_j = T_j . 1         (fwd-seeded through middle segment j)
  y_j = T_j^T . e_stop  (bwd-seeded; doubles as the exact tail for batches
                         whose L falls inside segment j via a src state
                         injected at tau=L)
  b = A^T G_{913} beta_{913} (exact backward chain over the last segment)

Products of >=114 positive transfer matrices are numerically rank-1
(Birkhoff contraction), so T_j w ~ x_j (y_j.w)/(y_j.1) and the partition
function composes from host-side dots telescoping across segments:
  Z ~ (b.x_{M-1}) prod_j [(y_j.x_{j-1})/(y_j.1)] (y_2.alpha)
truncated at the segment containing L.

Packing: the 2(M-1) half-chains form M-1=8 packs sharing one block [98,98]
bf16 stationary: P1=[alpha|y2], Pj=[x_j|y_{j+1}], P8=[x_8|b]. Packs are
grouped into TWO super-chains of 4 packs, each a [98, 256] datapath: one
PE matmul + one wide DVE Hadamard per slot (the wide TT amortizes the
fixed 125ns PSUM-access cost over 256 columns). The two super-chains are
independent and interleave on the engines, hiding cross-engine sync
latency. All chains run in the exponential domain (bf16, fp32 PSUM) with
host-precomputed per-(batch,step) shifts; the gold path score is a cheap
host gather.
"""
import numpy as np
import ml_dtypes
from contextlib import ExitStack

import concourse.bacc as bacc
import concourse.bass as bass
import concourse.tile as tile
from concourse import mybir
from concourse.bass_utils import run_bass_kernel_spmd

B, T, K = 512, 1024, 48
START, STOP = 46, 47
NEG = -10000.0
HOLD = 48
SRCL = 48
KF = 49
K2 = 98
NCORES = 8
BC = B // NCORES    # 64
M = 9               # segments
NP = M - 1          # packs (8)
NG = NP // 2        # packs per super-chain (4)
W = NG * BC         # super-chain width (256)
D = -(-1025 // M)   # 114 device slots
BND = [j * D for j in range(M)] + [1024]
CH = 38             # slots per emission chunk (114 = 3*38)

_nc_cache = {}


def _build_module(d_slots=D, ch=CH):
    key = ("nc", d_slots, ch)
    if key in _nc_cache:
        return _nc_cache[key]
    nc = bacc.Bacc(
        "TRN2",
        target_bir_lowering=False,
        debug=False,
        enable_asserts=False,
        num_devices=NCORES,
    )
    f32 = mybir.dt.float32
    bf16 = mybir.dt.bfloat16
    e_dram = nc.dram_tensor("etil", [K2, K2], bf16, kind="ExternalInput").ap()
    gA_dram = nc.dram_tensor("emisA", [K2, d_slots, W], bf16, kind="ExternalInput").ap()
    gB_dram = nc.dram_tensor("emisB", [K2, d_slots, W], bf16, kind="ExternalInput").ap()
    w0_dram = nc.dram_tensor("w0", [K2, 2 * W], bf16, kind="ExternalInput").ap()
    o_dram = nc.dram_tensor("uout", [K2, 2 * W], f32, kind="ExternalOutput").ap()

    with tile.TileContext(nc) as tc:
        with ExitStack() as ctx:
            const = ctx.enter_context(tc.tile_pool(name="const", bufs=1))
            wpool = ctx.enter_context(tc.tile_pool(name="wp", bufs=4))
            gexp_p = ctx.enter_context(tc.tile_pool(name="gexp", bufs=3))
            psum_p = ctx.enter_context(tc.tile_pool(name="ps", bufs=4, space="PSUM"))

            etile = const.tile([K2, K2], bf16)
            nc.gpsimd.dma_start(out=etile, in_=e_dram)

            wA = const.tile([K2, W], bf16)
            nc.gpsimd.dma_start(out=wA, in_=w0_dram[:, 0:W])
            wB = const.tile([K2, W], bf16)
            nc.gpsimd.dma_start(out=wB, in_=w0_dram[:, W : 2 * W])

            outA = const.tile([K2, W], f32)
            outB = const.tile([K2, W], f32)

            nstep = 0
            sched = [9, 12, 17] + [ch] * 100
            while nstep < d_slots:
                ns = min(sched.pop(0), d_slots - nstep)
                geA = gexp_p.tile([K2, ch, W], bf16, tag="geA")
                nc.sync.dma_start(
                    out=geA[:, :ns, :], in_=gA_dram[:, nstep : nstep + ns, :]
                )
                geB = gexp_p.tile([K2, ch, W], bf16, tag="geB")
                nc.scalar.dma_start(
                    out=geB[:, :ns, :], in_=gB_dram[:, nstep : nstep + ns, :]
                )
                for s in range(ns):
                    last = nstep + s == d_slots - 1
                    psA = psum_p.tile([K2, W], f32, tag="psA")
                    nc.tensor.matmul(psA, etile, wA, start=True, stop=True)
                    psB = psum_p.tile([K2, W], f32, tag="psB")
                    nc.tensor.matmul(psB, etile, wB, start=True, stop=True)
                    if last:
                        nc.vector.tensor_mul(outA, psA, geA[:, s, :])
                        nc.vector.tensor_mul(outB, psB, geB[:, s, :])
                    else:
                        wAn = wpool.tile([K2, W], bf16, tag="wA")
                        nc.vector.tensor_mul(wAn, psA, geA[:, s, :])
                        wA = wAn
                        wBn = wpool.tile([K2, W], bf16, tag="wB")
                        nc.vector.tensor_mul(wBn, psB, geB[:, s, :])
                        wB = wBn
                nstep += ns
            nc.sync.dma_start(out=o_dram[:, 0:W], in_=outA)
            nc.sync.dma_start(out=o_dram[:, W : 2 * W], in_=outB)

    nc.compile()
    _nc_cache[key] = nc
    return nc


def _host_prep(feats, seq_len, trans):
    feats = np.ascontiguousarray(feats, dtype=np.float32)
    seq_len = np.asarray(seq_len, dtype=np.int64)
    trans = np.asarray(trans, dtype=np.float32)
    L = seq_len

    mx = feats.max(axis=2)
    E64 = np.exp(trans.astype(np.float64)).T

    drift = []
    for b in range(6):
        fv = np.full(K, NEG, dtype=np.float64)
        fv[START] = 0.0
        Lb = int(L[b])
        for t in range(min(Lb, 256)):
            m = fv.max()
            wv = np.exp(fv - m)
            with np.errstate(divide="ignore"):
                fv = np.log(E64.T @ wv) + m + feats[b, t]
            drift.append((fv.max() - m) - mx[b, t])
    mu = float(np.mean(drift))

    driftb = []
    nb = 0
    for b in range(B):
        Lb = int(L[b])
        if Lb < 700:
            continue
        nb += 1
        if nb > 6:
            break
        bv = trans[STOP, :].astype(np.float64).copy()
        for t in range(Lb, Lb - 200, -1):
            m = bv.max()
            wv = np.exp(bv - m)
            gv = np.exp(feats[b, t - 1].astype(np.float64))
            with np.errstate(divide="ignore"):
                bv = np.log(E64 @ (gv * wv)) + m
            driftb.append((bv.max() - m) - mx[b, t - 1])
    mub = float(np.mean(driftb)) if driftb else mu

    c = mx + mu
    cb = mx + mub
    Ccum = np.cumsum(c, axis=1, dtype=np.float64)
    Ccumb = np.cumsum(cb, axis=1, dtype=np.float64)
    estop = np.exp(trans[STOP, :K].astype(np.float64))
    estop32 = estop.astype(np.float32)

    n_ = np.arange(1, D + 1)
    gfall = np.exp(feats - c[:, :, None])
    gball = np.exp(feats - cb[:, :, None])

    ga = np.zeros((B, D, KF), dtype=np.float32)
    livef = n_[None, :] <= L[:, None]
    ga[:, :, :K] = np.where(livef[:, :, None], gfall[:, :D, :], 0.0)
    holdon = n_[None, :] >= (L[:, None] + 1)
    ga[:, :, HOLD] = np.where(holdon, 1.0, 0.0)

    def x_rows(j):
        bjm1, bj = BND[j - 1], BND[j]
        act = L > bj
        tau = bjm1 + n_
        g = np.where(
            act[:, None, None], gfall[:, np.clip(tau - 1, 0, T - 1), :], 0.0
        ).astype(np.float32)
        return g, act

    def y_rows(j):
        bjm1, bj = BND[j - 1], BND[j]
        act = L >= bjm1 + 1
        sstar = np.maximum(bj - L, 0)
        g = np.zeros((B, D, KF), dtype=np.float32)
        tau = bj - n_
        valid = (
            (n_[None, :] >= np.maximum(sstar, 1)[:, None])
            & (n_[None, :] <= D - 1)
            & act[:, None]
        )
        gv = gball[:, np.clip(tau - 1, 0, T - 1), :]
        g[:, :, :K] = np.where(valid[:, :, None], gv, 0.0)
        g[act, D - 1, :K] = 1.0
        srcon = (n_[None, :] < sstar[:, None]) & act[:, None]
        g[:, :, SRCL] = np.where(srcon, 1.0, 0.0)
        u0 = np.zeros((B, KF), dtype=np.float32)
        inj = act & (sstar == 0)
        u0[inj, :K] = gball[inj, bj - 1, :] * estop32[None, :]
        u0[act & (sstar > 0), SRCL] = 1.0
        return g, u0

    def b_rows():
        bm1 = BND[M - 1]
        act = L >= bm1 + 1
        sstar = bm1 + 1 + D - L
        g = np.zeros((B, D, KF), dtype=np.float32)
        tau = bm1 + 1 + D - n_
        valid = (
            (n_[None, :] >= sstar[:, None]) & (n_[None, :] <= D - 1) & act[:, None]
        )
        gv = gball[:, np.clip(tau - 1, 0, T - 1), :]
        g[:, :, :K] = np.where(valid[:, :, None], gv, 0.0)
        g[act, D - 1, :K] = 1.0
        srcon = (n_[None, :] < sstar[:, None]) & act[:, None]
        g[:, :, SRCL] = np.where(srcon, 1.0, 0.0)
        u0 = np.zeros((B, KF), dtype=np.float32)
        u0[act, SRCL] = 1.0
        return g, u0

    xs = {}
    ys = {}
    acts = {}
    for j in range(2, M):
        xs[j], acts[j] = x_rows(j)
        ys[j] = y_rows(j)
    gb_, u0b = b_rows()

    def pack(gf, gbk):
        g = np.zeros((B, D, K2), dtype=np.float32)
        g[:, :, : gf.shape[2]] = gf
        g[:, :, KF : KF + gbk.shape[2]] = gbk
        return g

    P = [pack(ga, ys[2][0])]
    U = []
    u = np.zeros((B, K2), dtype=np.float32)
    u[:, START] = 1.0
    u[:, KF:] = ys[2][1]
    U.append(u)
    for j in range(2, M - 1):
        P.append(pack(xs[j], ys[j + 1][0]))
        u = np.zeros((B, K2), dtype=np.float32)
        u[acts[j], :K] = 1.0
        u[:, KF:] = ys[j + 1][1]
        U.append(u)
    P.append(pack(xs[M - 1], gb_))
    u = np.zeros((B, K2), dtype=np.float32)
    u[acts[M - 1], :K] = 1.0
    u[:, KF:] = u0b
    U.append(u)

    per_core = []
    for cix in range(NCORES):
        sl = slice(cix * BC, (cix + 1) * BC)
        gA = np.concatenate(
            [P[k][sl].transpose(2, 1, 0) for k in range(NG)], axis=2
        ).astype(ml_dtypes.bfloat16)
        gB = np.concatenate(
            [P[k][sl].transpose(2, 1, 0) for k in range(NG, NP)], axis=2
        ).astype(ml_dtypes.bfloat16)
        w0 = np.concatenate([U[k][sl].T for k in range(NP)], axis=1).astype(
            ml_dtypes.bfloat16
        )
        per_core.append(
            {
                "emisA": np.ascontiguousarray(gA),
                "emisB": np.ascontiguousarray(gB),
                "w0": np.ascontiguousarray(w0),
            }
        )

    S_ = np.zeros((K2, K2), dtype=np.float32)
    S_[:K, :K] = np.exp(trans).T
    S_[:K, HOLD] = estop32
    S_[HOLD, HOLD] = 1.0
    S_[KF : KF + K, KF : KF + K] = np.exp(trans)
    S_[KF + SRCL, KF : KF + K] = estop32
    S_[KF + SRCL, KF + SRCL] = 1.0
    etil = S_.astype(ml_dtypes.bfloat16)

    ar = np.arange(B)
    ctx = {
        "seq_len": L,
        "estop": estop,
        "C_at_L": Ccum[ar, L - 1],
        "Cal": Ccum[:, BND[1] - 1],
        "Cx": {j: Ccum[:, BND[j] - 1] - Ccum[:, BND[j - 1] - 1] for j in range(2, M)},
        "CyL": {j: Ccumb[ar, L - 1] - Ccumb[:, BND[j - 1] - 1] for j in range(2, M)},
        "Cb": Ccumb[ar, L - 1] - Ccumb[:, BND[M - 1] - 1],
    }
    return per_core, etil, ctx


def _combine(packs, ctx):
    """packs: list of NP arrays [K2, B] f64; returns per-batch scores."""
    L = ctx["seq_len"]
    estop = ctx["estop"]
    alpha = packs[0][:KF, :]
    xv = {}
    yv = {2: packs[0][KF : KF + K, :]}
    for j in range(2, M):
        xv[j] = packs[j - 1][:K, :]
    for j in range(2, M - 1):
        yv[j + 1] = packs[j - 1][KF : KF + K, :]
    bv = packs[NP - 1][KF : KF + K, :]

    d = {2: (yv[2] * alpha[:K, :]).sum(0)}
    n = {2: yv[2].sum(0)}
    for j in range(3, M):
        d[j] = (yv[j] * xv[j - 1]).sum(0)
        n[j] = yv[j].sum(0)
    d_b = (bv * xv[M - 1]).sum(0)

    scores = np.zeros(B)
    J1 = L <= BND[1] - 1
    scores[J1] = np.log(alpha[HOLD, J1]) + ctx["C_at_L"][J1]
    JM_ = L == BND[1]
    if JM_.any():
        dm = (alpha[:K, :] * estop[:, None]).sum(0)
        scores[JM_] = np.log(dm[JM_]) + ctx["Cal"][JM_]
    with np.errstate(divide="ignore", invalid="ignore"):
        for J in range(2, M):
            msk = (L > BND[J - 1]) & (L <= BND[J])
            sc = np.log(d[J]) + ctx["Cal"] + ctx["CyL"][J]
            for j in range(2, J):
                sc = sc + np.log(d[j]) - np.log(n[j]) + ctx["Cx"][j]
            scores[msk] = sc[msk]
        mskM = L > BND[M - 1]
        sc = np.log(d_b) + ctx["Cal"] + ctx["Cb"]
        for j in range(2, M):
            sc = sc + np.log(d[j]) - np.log(n[j]) + ctx["Cx"][j]
        scores[mskM] = sc[mskM]
    return scores


def _gold_score(feats, tags, seq_len, trans):
    feats = np.asarray(feats, dtype=np.float32)
    tags = np.asarray(tags, dtype=np.int64)
    seq_len = np.asarray(seq_len, dtype=np.int64)
    trans = np.asarray(trans, dtype=np.float32)
    tags_ext = np.concatenate(
        [np.full((B, 1), START, dtype=np.int64), tags], axis=1
    )
    trans_sc = trans[tags_ext[:, 1:], tags_ext[:, :-1]]
    emit_sc = np.take_along_axis(feats, tags_ext[:, 1:, None], axis=2)[..., 0]
    mask = np.arange(T)[None, :] < seq_len[:, None]
    last_tag = np.take_along_axis(tags_ext, seq_len[:, None], axis=1)[:, 0]
    gold = (
        np.where(mask, trans_sc + emit_sc, 0.0).sum(1, dtype=np.float64)
        + trans[STOP, last_tag]
    )
    return gold  # [B] f64


def kernel(feats, tags, seq_len, transitions):
    feats = np.asarray(feats)
    per_core, etil, ctx = _host_prep(feats, seq_len, transitions)
    nc = _build_module()
    in_maps = [{"etil": etil, **per_core[c]} for c in range(NCORES)]
    res = run_bass_kernel_spmd(nc, in_maps, list(range(NCORES)))
    outs = [np.asarray(res.results[c]["uout"]).astype(np.float64) for c in range(NCORES)]
    packs = [
        np.concatenate([o[:, k * BC : (k + 1) * BC] for o in outs], axis=1)
        for k in range(NP)
    ]
    scores = _combine(packs, ctx)
    gold = _gold_score(feats, tags, seq_len, transitions)
    loss = np.mean(scores - gold)
    return np.float32(loss)


# revision 23
# speedup vs baseline: 2.1868x; 1.0361x over previous
"""CRF negative log-likelihood loss on 8 Trainium2 NeuronCores.

Strategy: data-parallel over batch (64 sequences per core) with an
M=9-segment split of each sequence cutting the serial scan depth to 114
device slots. Boundaries BND[j] = j*114 (last segment 912..1024).

  alpha: exact forward chain over seg1 (48 states + hold for short L)
  x_j = T_j . 1         (fwd-seeded through middle segment j)
  y_j = T_j^T . e_stop  (bwd-seeded; doubles as the exact tail for batches
                         whose L falls inside segment j via a src state
                         injected at tau=L)
  b = A^T G_{913} beta_{913} (exact backward chain over the last segment)

Products of >=114 positive transfer matrices are numerically rank-1
(Birkhoff contraction), so T_j w ~ x_j (y_j.w)/(y_j.1) and the partition
function composes from host-side dots telescoping across segments:
  Z ~ (b.x_{M-1}) prod_j [(y_j.x_{j-1})/(y_j.1)] (y_2.alpha)
truncated at the segment containing L.

Packing: the 2(M-1) half-chains form M-1=8 packs sharing one block [98,98]
bf16 stationary: P1=[alpha|y2], Pj=[x_j|y_{j+1}], P8=[x_8|b]. Packs are
grouped into TWO super-chains of 4 packs, each a [98, 256] datapath: one
PE matmul + one wide DVE Hadamard per slot (the wide TT amortizes the
fixed 125ns PSUM-access cost over 256 columns). The two super-chains are
independent and interleave on the engines, hiding cross-engine sync
latency. All chains run in the exponential domain (bf16, fp32 PSUM) with
host-precomputed per-(batch,step) shifts; the gold path score is a cheap
host gather.
"""
import numpy as np
import ml_dtypes
from contextlib import ExitStack

import concourse.bacc as bacc
import concourse.bass as bass
import concourse.tile as tile
from concourse import mybir
from concourse.bass_utils import run_bass_kernel_spmd

B, T, K = 512, 1024, 48
START, STOP = 46, 47
NEG = -10000.0
HOLD = 48
SRCL = 48
KF = 49
K2 = 98
NCORES = 8
BC = B // NCORES    # 64
M = 9               # segments
NP = M - 1          # packs (8)
NG = NP // 2        # packs per super-chain (4)
W = NG * BC         # super-chain width (256)
D = -(-1025 // M)   # 114 device slots
BND = [j * D for j in range(M)] + [1024]
CH = 8              # slots per emission chunk (114 = 3*38)

_nc_cache = {}


def _build_module(d_slots=D, ch=CH):
    key = ("nc", d_slots, ch)
    if key in _nc_cache:
        return _nc_cache[key]
    nc = bacc.Bacc(
        "TRN2",
        target_bir_lowering=False,
        debug=False,
        enable_asserts=False,
        num_devices=NCORES,
    )
    f32 = mybir.dt.float32
    bf16 = mybir.dt.bfloat16
    e_dram = nc.dram_tensor("etil", [K2, K2], bf16, kind="ExternalInput").ap()
    gA_dram = nc.dram_tensor("emisA", [K2, d_slots, W], bf16, kind="ExternalInput").ap()
    gB_dram = nc.dram_tensor("emisB", [K2, d_slots, W], bf16, kind="ExternalInput").ap()
    w0_dram = nc.dram_tensor("w0", [K2, 2 * W], bf16, kind="ExternalInput").ap()
    o_dram = nc.dram_tensor("uout", [K2, 2 * W], f32, kind="ExternalOutput").ap()

    with tile.TileContext(nc) as tc:
        with ExitStack() as ctx:
            const = ctx.enter_context(tc.tile_pool(name="const", bufs=1))
            wpool = ctx.enter_context(tc.tile_pool(name="wp", bufs=4))
            gexp_p = ctx.enter_context(tc.tile_pool(name="gexp", bufs=3))
            psum_p = ctx.enter_context(tc.tile_pool(name="ps", bufs=2, space="PSUM"))

            etile = const.tile([K2, K2], bf16)
            nc.sync.dma_start(out=etile, in_=e_dram)

            wA = const.tile([K2, W], bf16)
            nc.sync.dma_start(out=wA, in_=w0_dram[:, 0:W])
            wB = const.tile([K2, W], bf16)
            nc.scalar.dma_start(out=wB, in_=w0_dram[:, W : 2 * W])

            outA = const.tile([K2, W], f32)
            outB = const.tile([K2, W], f32)

            nstep = 0
            sched = [2, 3, 5, 8] + [ch] * 100
            while nstep < d_slots:
                ns = min(sched.pop(0), d_slots - nstep)
                geA = gexp_p.tile([K2, ch, W], bf16, tag="geA")
                nc.sync.dma_start(
                    out=geA[:, :ns, :], in_=gA_dram[:, nstep : nstep + ns, :]
                )
                geB = gexp_p.tile([K2, ch, W], bf16, tag="geB")
                nc.scalar.dma_start(
                    out=geB[:, :ns, :], in_=gB_dram[:, nstep : nstep + ns, :]
                )
                for s in range(ns):
                    last = nstep + s == d_slots - 1
                    psA = psum_p.tile([K2, W], f32, tag="psA")
                    for c0 in range(0, W, 512):
                        c1 = min(c0 + 512, W)
                        nc.tensor.matmul(
                            psA[:, c0:c1], etile, wA[:, c0:c1], start=True, stop=True
                        )
                    psB = psum_p.tile([K2, W], f32, tag="psB")
                    for c0 in range(0, W, 512):
                        c1 = min(c0 + 512, W)
                        nc.tensor.matmul(
                            psB[:, c0:c1], etile, wB[:, c0:c1], start=True, stop=True
                        )
                    if last:
                        nc.vector.tensor_mul(outA, psA, geA[:, s, :])
                        nc.vector.tensor_mul(outB, psB, geB[:, s, :])
                    else:
                        wAn = wpool.tile([K2, W], bf16, tag="wA")
                        nc.vector.tensor_mul(wAn, psA, geA[:, s, :])
                        wA = wAn
                        wBn = wpool.tile([K2, W], bf16, tag="wB")
                        nc.vector.tensor_mul(wBn, psB, geB[:, s, :])
                        wB = wBn
                nstep += ns
            nc.sync.dma_start(out=o_dram[:, 0:W], in_=outA)
            nc.sync.dma_start(out=o_dram[:, W : 2 * W], in_=outB)

    nc.compile()
    _nc_cache[key] = nc
    return nc


def _host_prep(feats, seq_len, trans):
    feats = np.ascontiguousarray(feats, dtype=np.float32)
    seq_len = np.asarray(seq_len, dtype=np.int64)
    trans = np.asarray(trans, dtype=np.float32)
    L = seq_len

    mx = feats.max(axis=2)
    E64 = np.exp(trans.astype(np.float64)).T

    drift = []
    for b in range(6):
        fv = np.full(K, NEG, dtype=np.float64)
        fv[START] = 0.0
        Lb = int(L[b])
        for t in range(min(Lb, 256)):
            m = fv.max()
            wv = np.exp(fv - m)
            with np.errstate(divide="ignore"):
                fv = np.log(E64.T @ wv) + m + feats[b, t]
            drift.append((fv.max() - m) - mx[b, t])
    mu = float(np.mean(drift))

    driftb = []
    nb = 0
    for b in range(B):
        Lb = int(L[b])
        if Lb < 700:
            continue
        nb += 1
        if nb > 6:
            break
        bv = trans[STOP, :].astype(np.float64).copy()
        for t in range(Lb, Lb - 200, -1):
            m = bv.max()
            wv = np.exp(bv - m)
            gv = np.exp(feats[b, t - 1].astype(np.float64))
            with np.errstate(divide="ignore"):
                bv = np.log(E64 @ (gv * wv)) + m
            driftb.append((bv.max() - m) - mx[b, t - 1])
    mub = float(np.mean(driftb)) if driftb else mu

    c = mx + mu
    cb = mx + mub
    Ccum = np.cumsum(c, axis=1, dtype=np.float64)
    Ccumb = np.cumsum(cb, axis=1, dtype=np.float64)
    estop = np.exp(trans[STOP, :K].astype(np.float64))
    estop32 = estop.astype(np.float32)

    n_ = np.arange(1, D + 1)
    gfall = np.exp(feats - c[:, :, None])
    gball = np.exp(feats - cb[:, :, None])

    ga = np.zeros((B, D, KF), dtype=np.float32)
    livef = n_[None, :] <= L[:, None]
    ga[:, :, :K] = np.where(livef[:, :, None], gfall[:, :D, :], 0.0)
    holdon = n_[None, :] >= (L[:, None] + 1)
    ga[:, :, HOLD] = np.where(holdon, 1.0, 0.0)

    def x_rows(j):
        bjm1, bj = BND[j - 1], BND[j]
        act = L > bj
        tau = bjm1 + n_
        g = np.where(
            act[:, None, None], gfall[:, np.clip(tau - 1, 0, T - 1), :], 0.0
        ).astype(np.float32)
        return g, act

    def y_rows(j):
        bjm1, bj = BND[j - 1], BND[j]
        act = L >= bjm1 + 1
        sstar = np.maximum(bj - L, 0)
        g = np.zeros((B, D, KF), dtype=np.float32)
        tau = bj - n_
        valid = (
            (n_[None, :] >= np.maximum(sstar, 1)[:, None])
            & (n_[None, :] <= D - 1)
            & act[:, None]
        )
        gv = gball[:, np.clip(tau - 1, 0, T - 1), :]
        g[:, :, :K] = np.where(valid[:, :, None], gv, 0.0)
        g[act, D - 1, :K] = 1.0
        srcon = (n_[None, :] < sstar[:, None]) & act[:, None]
        g[:, :, SRCL] = np.where(srcon, 1.0, 0.0)
        u0 = np.zeros((B, KF), dtype=np.float32)
        inj = act & (sstar == 0)
        u0[inj, :K] = gball[inj, bj - 1, :] * estop32[None, :]
        u0[act & (sstar > 0), SRCL] = 1.0
        return g, u0

    def b_rows():
        bm1 = BND[M - 1]
        act = L >= bm1 + 1
        sstar = bm1 + 1 + D - L
        g = np.zeros((B, D, KF), dtype=np.float32)
        tau = bm1 + 1 + D - n_
        valid = (
            (n_[None, :] >= sstar[:, None]) & (n_[None, :] <= D - 1) & act[:, None]
        )
        gv = gball[:, np.clip(tau - 1, 0, T - 1), :]
        g[:, :, :K] = np.where(valid[:, :, None], gv, 0.0)
        g[act, D - 1, :K] = 1.0
        srcon = (n_[None, :] < sstar[:, None]) & act[:, None]
        g[:, :, SRCL] = np.where(srcon, 1.0, 0.0)
        u0 = np.zeros((B, KF), dtype=np.float32)
        u0[act, SRCL] = 1.0
        return g, u0

    xs = {}
    ys = {}
    acts = {}
    for j in range(2, M):
        xs[j], acts[j] = x_rows(j)
        ys[j] = y_rows(j)
    gb_, u0b = b_rows()

    def pack(gf, gbk):
        g = np.zeros((B, D, K2), dtype=np.float32)
        g[:, :, : gf.shape[2]] = gf
        g[:, :, KF : KF + gbk.shape[2]] = gbk
        return g

    P = [pack(ga, ys[2][0])]
    U = []
    u = np.zeros((B, K2), dtype=np.float32)
    u[:, START] = 1.0
    u[:, KF:] = ys[2][1]
    U.append(u)
    for j in range(2, M - 1):
        P.append(pack(xs[j], ys[j + 1][0]))
        u = np.zeros((B, K2), dtype=np.float32)
        u[acts[j], :K] = 1.0
        u[:, KF:] = ys[j + 1][1]
        U.append(u)
    P.append(pack(xs[M - 1], gb_))
    u = np.zeros((B, K2), dtype=np.float32)
    u[acts[M - 1], :K] = 1.0
    u[:, KF:] = u0b
    U.append(u)

    per_core = []
    for cix in range(NCORES):
        sl = slice(cix * BC, (cix + 1) * BC)
        gA = np.concatenate(
            [P[k][sl].transpose(2, 1, 0) for k in range(NG)], axis=2
        ).astype(ml_dtypes.bfloat16)
        gB = np.concatenate(
            [P[k][sl].transpose(2, 1, 0) for k in range(NG, NP)], axis=2
        ).astype(ml_dtypes.bfloat16)
        w0 = np.concatenate([U[k][sl].T for k in range(NP)], axis=1).astype(
            ml_dtypes.bfloat16
        )
        per_core.append(
            {
                "emisA": np.ascontiguousarray(gA),
                "emisB": np.ascontiguousarray(gB),
                "w0": np.ascontiguousarray(w0),
            }
        )

    S_ = np.zeros((K2, K2), dtype=np.float32)
    S_[:K, :K] = np.exp(trans).T
    S_[:K, HOLD] = estop32
    S_[HOLD, HOLD] = 1.0
    S_[KF : KF + K, KF : KF + K] = np.exp(trans)
    S_[KF + SRCL, KF : KF + K] = estop32
    S_[KF + SRCL, KF + SRCL] = 1.0
    etil = S_.astype(ml_dtypes.bfloat16)

    ar = np.arange(B)
    ctx = {
        "seq_len": L,
        "estop": estop,
        "C_at_L": Ccum[ar, L - 1],
        "Cal": Ccum[:, BND[1] - 1],
        "Cx": {j: Ccum[:, BND[j] - 1] - Ccum[:, BND[j - 1] - 1] for j in range(2, M)},
        "CyL": {j: Ccumb[ar, L - 1] - Ccumb[:, BND[j - 1] - 1] for j in range(2, M)},
        "Cb": Ccumb[ar, L - 1] - Ccumb[:, BND[M - 1] - 1],
    }
    return per_core, etil, ctx


def _combine(packs, ctx):
    """packs: list of NP arrays [K2, B] f64; returns per-batch scores."""
    L = ctx["seq_len"]
    estop = ctx["estop"]
    alpha = packs[0][:KF, :]
    xv = {}
    yv = {2: packs[0][KF : KF + K, :]}
    for j in range(2, M):
        xv[j] = packs[j - 1][:K, :]
    for j in range(2, M - 1):
        yv[j + 1] = packs[j - 1][KF : KF + K, :]
    bv = packs[NP - 1][KF : KF + K, :]

    d = {2: (yv[2] * alpha[:K, :]).sum(0)}
    n = {2: yv[2].sum(0)}
    for j in range(3, M):
        d[j] = (yv[j] * xv[j - 1]).sum(0)
        n[j] = yv[j].sum(0)
    d_b = (bv * xv[M - 1]).sum(0)

    scores = np.zeros(B)
    J1 = L <= BND[1] - 1
    scores[J1] = np.log(alpha[HOLD, J1]) + ctx["C_at_L"][J1]
    JM_ = L == BND[1]
    if JM_.any():
        dm = (alpha[:K, :] * estop[:, None]).sum(0)
        scores[JM_] = np.log(dm[JM_]) + ctx["Cal"][JM_]
    with np.errstate(divide="ignore", invalid="ignore"):
        for J in range(2, M):
            msk = (L > BND[J - 1]) & (L <= BND[J])
            sc = np.log(d[J]) + ctx["Cal"] + ctx["CyL"][J]
            for j in range(2, J):
                sc = sc + np.log(d[j]) - np.log(n[j]) + ctx["Cx"][j]
            scores[msk] = sc[msk]
        mskM = L > BND[M - 1]
        sc = np.log(d_b) + ctx["Cal"] + ctx["Cb"]
        for j in range(2, M):
            sc = sc + np.log(d[j]) - np.log(n[j]) + ctx["Cx"][j]
        scores[mskM] = sc[mskM]
    return scores


def _gold_score(feats, tags, seq_len, trans):
    feats = np.asarray(feats, dtype=np.float32)
    tags = np.asarray(tags, dtype=np.int64)
    seq_len = np.asarray(seq_len, dtype=np.int64)
    trans = np.asarray(trans, dtype=np.float32)
    tags_ext = np.concatenate(
        [np.full((B, 1), START, dtype=np.int64), tags], axis=1
    )
    trans_sc = trans[tags_ext[:, 1:], tags_ext[:, :-1]]
    emit_sc = np.take_along_axis(feats, tags_ext[:, 1:, None], axis=2)[..., 0]
    mask = np.arange(T)[None, :] < seq_len[:, None]
    last_tag = np.take_along_axis(tags_ext, seq_len[:, None], axis=1)[:, 0]
    gold = (
        np.where(mask, trans_sc + emit_sc, 0.0).sum(1, dtype=np.float64)
        + trans[STOP, last_tag]
    )
    return gold  # [B] f64


def kernel(feats, tags, seq_len, transitions):
    feats = np.asarray(feats)
    per_core, etil, ctx = _host_prep(feats, seq_len, transitions)
    nc = _build_module()
    in_maps = [{"etil": etil, **per_core[c]} for c in range(NCORES)]
    res = run_bass_kernel_spmd(nc, in_maps, list(range(NCORES)))
    outs = [np.asarray(res.results[c]["uout"]).astype(np.float64) for c in range(NCORES)]
    packs = [
        np.concatenate([o[:, k * BC : (k + 1) * BC] for o in outs], axis=1)
        for k in range(NP)
    ]
    scores = _combine(packs, ctx)
    gold = _gold_score(feats, tags, seq_len, transitions)
    loss = np.mean(scores - gold)
    return np.float32(loss)


# revision 24
# speedup vs baseline: 2.4826x; 1.1352x over previous
"""CRF negative log-likelihood loss on 8 Trainium2 NeuronCores.

Strategy: data-parallel over batch (64 sequences per core) with an
M=9-segment split of each sequence cutting the serial scan depth to 114
device slots. Boundaries BND[j] = j*114 (last segment 912..1024).

  alpha: exact forward chain over seg1 (48 states + hold for short L)
  x_j = T_j . 1         (fwd-seeded through middle segment j)
  y_j = T_j^T . e_stop  (bwd-seeded; doubles as the exact tail for batches
                         whose L falls inside segment j via a src state
                         injected at tau=L)
  b = A^T G_{913} beta_{913} (exact backward chain over the last segment)

Products of >=114 positive transfer matrices are numerically rank-1
(Birkhoff contraction), so T_j w ~ x_j (y_j.w)/(y_j.1) and the partition
function composes from host-side dots telescoping across segments:
  Z ~ (b.x_{M-1}) prod_j [(y_j.x_{j-1})/(y_j.1)] (y_2.alpha)
truncated at the segment containing L.

Packing: the 2(M-1) half-chains form M-1=8 packs sharing one block [98,98]
bf16 stationary: P1=[alpha|y2], Pj=[x_j|y_{j+1}], P8=[x_8|b]. Packs are
grouped into TWO super-chains of 4 packs, each a [98, 256] datapath: one
PE matmul + one wide DVE Hadamard per slot (the wide TT amortizes the
fixed 125ns PSUM-access cost over 256 columns). The two super-chains are
independent and interleave on the engines, hiding cross-engine sync
latency. All chains run in the exponential domain (bf16, fp32 PSUM) with
host-precomputed per-(batch,step) shifts; the gold path score is a cheap
host gather.
"""
import numpy as np
import ml_dtypes
from contextlib import ExitStack

import concourse.bacc as bacc
import concourse.bass as bass
import concourse.tile as tile
from concourse import mybir
from concourse.bass_utils import run_bass_kernel_spmd

B, T, K = 512, 1024, 48
START, STOP = 46, 47
NEG = -10000.0
HOLD = 48
SRCL = 48
KF = 49
K2 = 98
NCORES = 8
BC = B // NCORES    # 64
M = 9               # segments
NP = M - 1          # packs (8)
NG = NP // 2        # packs per super-chain (4)
W = NG * BC         # super-chain width (256)
D = -(-1025 // M)   # 114 device slots
BND = [j * D for j in range(M)] + [1024]
CH = 8              # slots per emission chunk (114 = 3*38)

_nc_cache = {}


def _build_module(d_slots=D, ch=CH):
    key = ("nc", d_slots, ch)
    if key in _nc_cache:
        return _nc_cache[key]
    nc = bacc.Bacc(
        "TRN2",
        target_bir_lowering=False,
        debug=False,
        enable_asserts=False,
        num_devices=NCORES,
    )
    f32 = mybir.dt.float32
    bf16 = mybir.dt.bfloat16
    e_dram = nc.dram_tensor("etil", [K2, K2], bf16, kind="ExternalInput").ap()
    gA_dram = nc.dram_tensor("emisA", [K2, d_slots, W], bf16, kind="ExternalInput").ap()
    gB_dram = nc.dram_tensor("emisB", [K2, d_slots, W], bf16, kind="ExternalInput").ap()
    w0_dram = nc.dram_tensor("w0", [K2, 2 * W], bf16, kind="ExternalInput").ap()
    o_dram = nc.dram_tensor("uout", [K2, 2 * W], bf16, kind="ExternalOutput").ap()

    with tile.TileContext(nc) as tc:
        with ExitStack() as ctx:
            const = ctx.enter_context(tc.tile_pool(name="const", bufs=1))
            wpool = ctx.enter_context(tc.tile_pool(name="wp", bufs=4))
            gexp_p = ctx.enter_context(tc.tile_pool(name="gexp", bufs=3))
            psum_p = ctx.enter_context(tc.tile_pool(name="ps", bufs=2, space="PSUM"))

            etile = const.tile([K2, K2], bf16)
            nc.sync.dma_start(out=etile, in_=e_dram)

            wA = const.tile([K2, W], bf16)
            nc.sync.dma_start(out=wA, in_=w0_dram[:, 0:W])
            wB = const.tile([K2, W], bf16)
            nc.scalar.dma_start(out=wB, in_=w0_dram[:, W : 2 * W])

            outT = const.tile([K2, 2 * W], bf16)
            outA = outT[:, 0:W]
            outB = outT[:, W : 2 * W]

            nstep = 0
            sched = [2, 3, 5, 8] + [ch] * 100
            while nstep < d_slots:
                ns = min(sched.pop(0), d_slots - nstep)
                geA = gexp_p.tile([K2, ch, W], bf16, tag="geA")
                nc.sync.dma_start(
                    out=geA[:, :ns, :], in_=gA_dram[:, nstep : nstep + ns, :]
                )
                geB = gexp_p.tile([K2, ch, W], bf16, tag="geB")
                nc.scalar.dma_start(
                    out=geB[:, :ns, :], in_=gB_dram[:, nstep : nstep + ns, :]
                )
                for s in range(ns):
                    last = nstep + s == d_slots - 1
                    psA = psum_p.tile([K2, W], f32, tag="psA")
                    for c0 in range(0, W, 512):
                        c1 = min(c0 + 512, W)
                        nc.tensor.matmul(
                            psA[:, c0:c1], etile, wA[:, c0:c1], start=True, stop=True
                        )
                    psB = psum_p.tile([K2, W], f32, tag="psB")
                    for c0 in range(0, W, 512):
                        c1 = min(c0 + 512, W)
                        nc.tensor.matmul(
                            psB[:, c0:c1], etile, wB[:, c0:c1], start=True, stop=True
                        )
                    if last:
                        nc.vector.tensor_mul(outA, psA, geA[:, s, :])
                        nc.vector.tensor_mul(outB, psB, geB[:, s, :])
                    else:
                        wAn = wpool.tile([K2, W], bf16, tag="wA")
                        nc.vector.tensor_mul(wAn, psA, geA[:, s, :])
                        wA = wAn
                        wBn = wpool.tile([K2, W], bf16, tag="wB")
                        nc.vector.tensor_mul(wBn, psB, geB[:, s, :])
                        wB = wBn
                nstep += ns
            nc.sync.dma_start(out=o_dram, in_=outT)

    nc.compile()
    _nc_cache[key] = nc
    return nc


def _host_prep(feats, seq_len, trans):
    feats = np.ascontiguousarray(feats, dtype=np.float32)
    seq_len = np.asarray(seq_len, dtype=np.int64)
    trans = np.asarray(trans, dtype=np.float32)
    L = seq_len

    mx = feats.max(axis=2)
    E64 = np.exp(trans.astype(np.float64)).T

    drift = []
    for b in range(6):
        fv = np.full(K, NEG, dtype=np.float64)
        fv[START] = 0.0
        Lb = int(L[b])
        for t in range(min(Lb, 256)):
            m = fv.max()
            wv = np.exp(fv - m)
            with np.errstate(divide="ignore"):
                fv = np.log(E64.T @ wv) + m + feats[b, t]
            drift.append((fv.max() - m) - mx[b, t])
    mu = float(np.mean(drift))

    driftb = []
    nb = 0
    for b in range(B):
        Lb = int(L[b])
        if Lb < 700:
            continue
        nb += 1
        if nb > 6:
            break
        bv = trans[STOP, :].astype(np.float64).copy()
        for t in range(Lb, Lb - 200, -1):
            m = bv.max()
            wv = np.exp(bv - m)
            gv = np.exp(feats[b, t - 1].astype(np.float64))
            with np.errstate(divide="ignore"):
                bv = np.log(E64 @ (gv * wv)) + m
            driftb.append((bv.max() - m) - mx[b, t - 1])
    mub = float(np.mean(driftb)) if driftb else mu

    c = mx + mu
    cb = mx + mub
    Ccum = np.cumsum(c, axis=1, dtype=np.float64)
    Ccumb = np.cumsum(cb, axis=1, dtype=np.float64)
    estop = np.exp(trans[STOP, :K].astype(np.float64))
    estop32 = estop.astype(np.float32)

    n_ = np.arange(1, D + 1)
    gfall = np.exp(feats - c[:, :, None])
    gball = np.exp(feats - cb[:, :, None])

    ga = np.zeros((B, D, KF), dtype=np.float32)
    livef = n_[None, :] <= L[:, None]
    ga[:, :, :K] = np.where(livef[:, :, None], gfall[:, :D, :], 0.0)
    holdon = n_[None, :] >= (L[:, None] + 1)
    ga[:, :, HOLD] = np.where(holdon, 1.0, 0.0)

    def x_rows(j):
        bjm1, bj = BND[j - 1], BND[j]
        act = L > bj
        tau = bjm1 + n_
        g = np.where(
            act[:, None, None], gfall[:, np.clip(tau - 1, 0, T - 1), :], 0.0
        ).astype(np.float32)
        return g, act

    def y_rows(j):
        bjm1, bj = BND[j - 1], BND[j]
        act = L >= bjm1 + 1
        sstar = np.maximum(bj - L, 0)
        g = np.zeros((B, D, KF), dtype=np.float32)
        tau = bj - n_
        valid = (
            (n_[None, :] >= np.maximum(sstar, 1)[:, None])
            & (n_[None, :] <= D - 1)
            & act[:, None]
        )
        gv = gball[:, np.clip(tau - 1, 0, T - 1), :]
        g[:, :, :K] = np.where(valid[:, :, None], gv, 0.0)
        g[act, D - 1, :K] = 1.0
        srcon = (n_[None, :] < sstar[:, None]) & act[:, None]
        g[:, :, SRCL] = np.where(srcon, 1.0, 0.0)
        u0 = np.zeros((B, KF), dtype=np.float32)
        inj = act & (sstar == 0)
        u0[inj, :K] = gball[inj, bj - 1, :] * estop32[None, :]
        u0[act & (sstar > 0), SRCL] = 1.0
        return g, u0

    def b_rows():
        bm1 = BND[M - 1]
        act = L >= bm1 + 1
        sstar = bm1 + 1 + D - L
        g = np.zeros((B, D, KF), dtype=np.float32)
        tau = bm1 + 1 + D - n_
        valid = (
            (n_[None, :] >= sstar[:, None]) & (n_[None, :] <= D - 1) & act[:, None]
        )
        gv = gball[:, np.clip(tau - 1, 0, T - 1), :]
        g[:, :, :K] = np.where(valid[:, :, None], gv, 0.0)
        g[act, D - 1, :K] = 1.0
        srcon = (n_[None, :] < sstar[:, None]) & act[:, None]
        g[:, :, SRCL] = np.where(srcon, 1.0, 0.0)
        u0 = np.zeros((B, KF), dtype=np.float32)
        u0[act, SRCL] = 1.0
        return g, u0

    xs = {}
    ys = {}
    acts = {}
    for j in range(2, M):
        xs[j], acts[j] = x_rows(j)
        ys[j] = y_rows(j)
    gb_, u0b = b_rows()

    def pack(gf, gbk):
        g = np.zeros((B, D, K2), dtype=np.float32)
        g[:, :, : gf.shape[2]] = gf
        g[:, :, KF : KF + gbk.shape[2]] = gbk
        return g

    P = [pack(ga, ys[2][0])]
    U = []
    u = np.zeros((B, K2), dtype=np.float32)
    u[:, START] = 1.0
    u[:, KF:] = ys[2][1]
    U.append(u)
    for j in range(2, M - 1):
        P.append(pack(xs[j], ys[j + 1][0]))
        u = np.zeros((B, K2), dtype=np.float32)
        u[acts[j], :K] = 1.0
        u[:, KF:] = ys[j + 1][1]
        U.append(u)
    P.append(pack(xs[M - 1], gb_))
    u = np.zeros((B, K2), dtype=np.float32)
    u[acts[M - 1], :K] = 1.0
    u[:, KF:] = u0b
    U.append(u)

    per_core = []
    for cix in range(NCORES):
        sl = slice(cix * BC, (cix + 1) * BC)
        gA = np.concatenate(
            [P[k][sl].transpose(2, 1, 0) for k in range(NG)], axis=2
        ).astype(ml_dtypes.bfloat16)
        gB = np.concatenate(
            [P[k][sl].transpose(2, 1, 0) for k in range(NG, NP)], axis=2
        ).astype(ml_dtypes.bfloat16)
        w0 = np.concatenate([U[k][sl].T for k in range(NP)], axis=1).astype(
            ml_dtypes.bfloat16
        )
        per_core.append(
            {
                "emisA": np.ascontiguousarray(gA),
                "emisB": np.ascontiguousarray(gB),
                "w0": np.ascontiguousarray(w0),
            }
        )

    S_ = np.zeros((K2, K2), dtype=np.float32)
    S_[:K, :K] = np.exp(trans).T
    S_[:K, HOLD] = estop32
    S_[HOLD, HOLD] = 1.0
    S_[KF : KF + K, KF : KF + K] = np.exp(trans)
    S_[KF + SRCL, KF : KF + K] = estop32
    S_[KF + SRCL, KF + SRCL] = 1.0
    etil = S_.astype(ml_dtypes.bfloat16)

    ar = np.arange(B)
    ctx = {
        "seq_len": L,
        "estop": estop,
        "C_at_L": Ccum[ar, L - 1],
        "Cal": Ccum[:, BND[1] - 1],
        "Cx": {j: Ccum[:, BND[j] - 1] - Ccum[:, BND[j - 1] - 1] for j in range(2, M)},
        "CyL": {j: Ccumb[ar, L - 1] - Ccumb[:, BND[j - 1] - 1] for j in range(2, M)},
        "Cb": Ccumb[ar, L - 1] - Ccumb[:, BND[M - 1] - 1],
    }
    return per_core, etil, ctx


def _combine(packs, ctx):
    """packs: list of NP arrays [K2, B] f64; returns per-batch scores."""
    L = ctx["seq_len"]
    estop = ctx["estop"]
    alpha = packs[0][:KF, :]
    xv = {}
    yv = {2: packs[0][KF : KF + K, :]}
    for j in range(2, M):
        xv[j] = packs[j - 1][:K, :]
    for j in range(2, M - 1):
        yv[j + 1] = packs[j - 1][KF : KF + K, :]
    bv = packs[NP - 1][KF : KF + K, :]

    d = {2: (yv[2] * alpha[:K, :]).sum(0)}
    n = {2: yv[2].sum(0)}
    for j in range(3, M):
        d[j] = (yv[j] * xv[j - 1]).sum(0)
        n[j] = yv[j].sum(0)
    d_b = (bv * xv[M - 1]).sum(0)

    scores = np.zeros(B)
    J1 = L <= BND[1] - 1
    scores[J1] = np.log(alpha[HOLD, J1]) + ctx["C_at_L"][J1]
    JM_ = L == BND[1]
    if JM_.any():
        dm = (alpha[:K, :] * estop[:, None]).sum(0)
        scores[JM_] = np.log(dm[JM_]) + ctx["Cal"][JM_]
    with np.errstate(divide="ignore", invalid="ignore"):
        for J in range(2, M):
            msk = (L > BND[J - 1]) & (L <= BND[J])
            sc = np.log(d[J]) + ctx["Cal"] + ctx["CyL"][J]
            for j in range(2, J):
                sc = sc + np.log(d[j]) - np.log(n[j]) + ctx["Cx"][j]
            scores[msk] = sc[msk]
        mskM = L > BND[M - 1]
        sc = np.log(d_b) + ctx["Cal"] + ctx["Cb"]
        for j in range(2, M):
            sc = sc + np.log(d[j]) - np.log(n[j]) + ctx["Cx"][j]
        scores[mskM] = sc[mskM]
    return scores


def _gold_score(feats, tags, seq_len, trans):
    feats = np.asarray(feats, dtype=np.float32)
    tags = np.asarray(tags, dtype=np.int64)
    seq_len = np.asarray(seq_len, dtype=np.int64)
    trans = np.asarray(trans, dtype=np.float32)
    tags_ext = np.concatenate(
        [np.full((B, 1), START, dtype=np.int64), tags], axis=1
    )
    trans_sc = trans[tags_ext[:, 1:], tags_ext[:, :-1]]
    emit_sc = np.take_along_axis(feats, tags_ext[:, 1:, None], axis=2)[..., 0]
    mask = np.arange(T)[None, :] < seq_len[:, None]
    last_tag = np.take_along_axis(tags_ext, seq_len[:, None], axis=1)[:, 0]
    gold = (
        np.where(mask, trans_sc + emit_sc, 0.0).sum(1, dtype=np.float64)
        + trans[STOP, last_tag]
    )
    return gold  # [B] f64


def kernel(feats, tags, seq_len, transitions):
    feats = np.asarray(feats)
    per_core, etil, ctx = _host_prep(feats, seq_len, transitions)
    nc = _build_module()
    in_maps = [{"etil": etil, **per_core[c]} for c in range(NCORES)]
    res = run_bass_kernel_spmd(nc, in_maps, list(range(NCORES)))
    outs = [np.asarray(res.results[c]["uout"]).astype(np.float64) for c in range(NCORES)]
    packs = [
        np.concatenate([o[:, k * BC : (k + 1) * BC] for o in outs], axis=1)
        for k in range(NP)
    ]
    scores = _combine(packs, ctx)
    gold = _gold_score(feats, tags, seq_len, transitions)
    loss = np.mean(scores - gold)
    return np.float32(loss)


# revision 25
# speedup vs baseline: 2.6973x; 1.0865x over previous
"""CRF negative log-likelihood loss on 8 Trainium2 NeuronCores.

Strategy: data-parallel over batch (64 sequences per core) with an
M=9-segment split of each sequence cutting the serial scan depth to 114
device slots. Boundaries BND[j] = j*114 (last segment 912..1024).

  alpha: exact forward chain over seg1 (48 states + hold for short L)
  x_j = T_j . 1         (fwd-seeded through middle segment j)
  y_j = T_j^T . e_stop  (bwd-seeded; doubles as the exact tail for batches
                         whose L falls inside segment j via a src state
                         injected at tau=L)
  b = A^T G_{913} beta_{913} (exact backward chain over the last segment)

Products of >=114 positive transfer matrices are numerically rank-1
(Birkhoff contraction), so T_j w ~ x_j (y_j.w)/(y_j.1) and the partition
function composes from host-side dots telescoping across segments:
  Z ~ (b.x_{M-1}) prod_j [(y_j.x_{j-1})/(y_j.1)] (y_2.alpha)
truncated at the segment containing L.

Packing: the 2(M-1) half-chains form M-1=8 packs sharing one block [98,98]
bf16 stationary: P1=[alpha|y2], Pj=[x_j|y_{j+1}], P8=[x_8|b]. Packs are
grouped into TWO super-chains of 4 packs, each a [98, 256] datapath: one
PE matmul + one wide DVE Hadamard per slot (the wide TT amortizes the
fixed 125ns PSUM-access cost over 256 columns). The two super-chains are
independent and interleave on the engines, hiding cross-engine sync
latency. All chains run in the exponential domain (bf16, fp32 PSUM) with
host-precomputed per-(batch,step) shifts; the gold path score is a cheap
host gather.
"""
import numpy as np
import ml_dtypes
from contextlib import ExitStack

import concourse.bacc as bacc
import concourse.bass as bass
import concourse.tile as tile
from concourse import mybir
from concourse.bass_utils import run_bass_kernel_spmd

B, T, K = 512, 1024, 48
START, STOP = 46, 47
NEG = -10000.0
HOLD = 48
SRCL = 48
KF = 49
K2 = 98
NCORES = 8
BC = B // NCORES    # 64
M = 9               # segments
NP = M - 1          # packs (8)
NG = NP // 2        # packs per super-chain (4)
W = NG * BC         # super-chain width (256)
D = -(-1025 // M)   # 114 device slots
BND = [j * D for j in range(M)] + [1024]
CH = 8              # slots per emission chunk (114 = 3*38)

_nc_cache = {}


def _build_module(d_slots=D, ch=CH):
    key = ("nc", d_slots, ch)
    if key in _nc_cache:
        return _nc_cache[key]
    nc = bacc.Bacc(
        "TRN2",
        target_bir_lowering=False,
        debug=False,
        enable_asserts=False,
        num_devices=NCORES,
    )
    f32 = mybir.dt.float32
    bf16 = mybir.dt.bfloat16
    fp8 = mybir.dt.float8e4
    e_dram = nc.dram_tensor("etil", [K2, K2], bf16, kind="ExternalInput").ap()
    gA_dram = nc.dram_tensor("emisA", [K2, d_slots, W], fp8, kind="ExternalInput").ap()
    gB_dram = nc.dram_tensor("emisB", [K2, d_slots, W], fp8, kind="ExternalInput").ap()
    w0_dram = nc.dram_tensor("w0", [K2, 2 * W], bf16, kind="ExternalInput").ap()
    o_dram = nc.dram_tensor("uout", [K2, 2 * W], bf16, kind="ExternalOutput").ap()

    with tile.TileContext(nc) as tc:
        with ExitStack() as ctx:
            const = ctx.enter_context(tc.tile_pool(name="const", bufs=1))
            wpool = ctx.enter_context(tc.tile_pool(name="wp", bufs=4))
            gexp_p = ctx.enter_context(tc.tile_pool(name="gexp", bufs=3))
            psum_p = ctx.enter_context(tc.tile_pool(name="ps", bufs=2, space="PSUM"))

            etile = const.tile([K2, K2], bf16)
            nc.sync.dma_start(out=etile, in_=e_dram)

            wA = const.tile([K2, W], bf16)
            nc.sync.dma_start(out=wA, in_=w0_dram[:, 0:W])
            wB = const.tile([K2, W], bf16)
            nc.scalar.dma_start(out=wB, in_=w0_dram[:, W : 2 * W])

            outT = const.tile([K2, 2 * W], bf16)
            outA = outT[:, 0:W]
            outB = outT[:, W : 2 * W]

            nstep = 0
            sched = [2, 3, 5, 8] + [ch] * 100
            while nstep < d_slots:
                ns = min(sched.pop(0), d_slots - nstep)
                geA = gexp_p.tile([K2, ch, W], fp8, tag="geA")
                nc.sync.dma_start(
                    out=geA[:, :ns, :], in_=gA_dram[:, nstep : nstep + ns, :]
                )
                geB = gexp_p.tile([K2, ch, W], fp8, tag="geB")
                nc.scalar.dma_start(
                    out=geB[:, :ns, :], in_=gB_dram[:, nstep : nstep + ns, :]
                )
                for s in range(ns):
                    last = nstep + s == d_slots - 1
                    psA = psum_p.tile([K2, W], f32, tag="psA")
                    for c0 in range(0, W, 512):
                        c1 = min(c0 + 512, W)
                        nc.tensor.matmul(
                            psA[:, c0:c1], etile, wA[:, c0:c1], start=True, stop=True
                        )
                    psB = psum_p.tile([K2, W], f32, tag="psB")
                    for c0 in range(0, W, 512):
                        c1 = min(c0 + 512, W)
                        nc.tensor.matmul(
                            psB[:, c0:c1], etile, wB[:, c0:c1], start=True, stop=True
                        )
                    if last:
                        nc.vector.tensor_mul(outA, psA, geA[:, s, :])
                        nc.vector.tensor_mul(outB, psB, geB[:, s, :])
                    else:
                        wAn = wpool.tile([K2, W], bf16, tag="wA")
                        nc.vector.tensor_mul(wAn, psA, geA[:, s, :])
                        wA = wAn
                        wBn = wpool.tile([K2, W], bf16, tag="wB")
                        nc.vector.tensor_mul(wBn, psB, geB[:, s, :])
                        wB = wBn
                nstep += ns
            nc.sync.dma_start(out=o_dram, in_=outT)

    nc.compile()
    _nc_cache[key] = nc
    return nc


def _host_prep(feats, seq_len, trans):
    feats = np.ascontiguousarray(feats, dtype=np.float32)
    seq_len = np.asarray(seq_len, dtype=np.int64)
    trans = np.asarray(trans, dtype=np.float32)
    L = seq_len

    mx = feats.max(axis=2)
    E64 = np.exp(trans.astype(np.float64)).T

    drift = []
    for b in range(6):
        fv = np.full(K, NEG, dtype=np.float64)
        fv[START] = 0.0
        Lb = int(L[b])
        for t in range(min(Lb, 256)):
            m = fv.max()
            wv = np.exp(fv - m)
            with np.errstate(divide="ignore"):
                fv = np.log(E64.T @ wv) + m + feats[b, t]
            drift.append((fv.max() - m) - mx[b, t])
    mu = float(np.mean(drift))

    driftb = []
    nb = 0
    for b in range(B):
        Lb = int(L[b])
        if Lb < 700:
            continue
        nb += 1
        if nb > 6:
            break
        bv = trans[STOP, :].astype(np.float64).copy()
        for t in range(Lb, Lb - 200, -1):
            m = bv.max()
            wv = np.exp(bv - m)
            gv = np.exp(feats[b, t - 1].astype(np.float64))
            with np.errstate(divide="ignore"):
                bv = np.log(E64 @ (gv * wv)) + m
            driftb.append((bv.max() - m) - mx[b, t - 1])
    mub = float(np.mean(driftb)) if driftb else mu

    c = mx + mu
    cb = mx + mub
    Ccum = np.cumsum(c, axis=1, dtype=np.float64)
    Ccumb = np.cumsum(cb, axis=1, dtype=np.float64)
    estop = np.exp(trans[STOP, :K].astype(np.float64))
    estop32 = estop.astype(np.float32)

    n_ = np.arange(1, D + 1)
    gfall = np.exp(feats - c[:, :, None])
    gball = np.exp(feats - cb[:, :, None])

    ga = np.zeros((B, D, KF), dtype=np.float32)
    livef = n_[None, :] <= L[:, None]
    ga[:, :, :K] = np.where(livef[:, :, None], gfall[:, :D, :], 0.0)
    holdon = n_[None, :] >= (L[:, None] + 1)
    ga[:, :, HOLD] = np.where(holdon, 1.0, 0.0)

    def x_rows(j):
        bjm1, bj = BND[j - 1], BND[j]
        act = L > bj
        tau = bjm1 + n_
        g = np.where(
            act[:, None, None], gfall[:, np.clip(tau - 1, 0, T - 1), :], 0.0
        ).astype(np.float32)
        return g, act

    def y_rows(j):
        bjm1, bj = BND[j - 1], BND[j]
        act = L >= bjm1 + 1
        sstar = np.maximum(bj - L, 0)
        g = np.zeros((B, D, KF), dtype=np.float32)
        tau = bj - n_
        valid = (
            (n_[None, :] >= np.maximum(sstar, 1)[:, None])
            & (n_[None, :] <= D - 1)
            & act[:, None]
        )
        gv = gball[:, np.clip(tau - 1, 0, T - 1), :]
        g[:, :, :K] = np.where(valid[:, :, None], gv, 0.0)
        g[act, D - 1, :K] = 1.0
        srcon = (n_[None, :] < sstar[:, None]) & act[:, None]
        g[:, :, SRCL] = np.where(srcon, 1.0, 0.0)
        u0 = np.zeros((B, KF), dtype=np.float32)
        inj = act & (sstar == 0)
        u0[inj, :K] = gball[inj, bj - 1, :] * estop32[None, :]
        u0[act & (sstar > 0), SRCL] = 1.0
        return g, u0

    def b_rows():
        bm1 = BND[M - 1]
        act = L >= bm1 + 1
        sstar = bm1 + 1 + D - L
        g = np.zeros((B, D, KF), dtype=np.float32)
        tau = bm1 + 1 + D - n_
        valid = (
            (n_[None, :] >= sstar[:, None]) & (n_[None, :] <= D - 1) & act[:, None]
        )
        gv = gball[:, np.clip(tau - 1, 0, T - 1), :]
        g[:, :, :K] = np.where(valid[:, :, None], gv, 0.0)
        g[act, D - 1, :K] = 1.0
        srcon = (n_[None, :] < sstar[:, None]) & act[:, None]
        g[:, :, SRCL] = np.where(srcon, 1.0, 0.0)
        u0 = np.zeros((B, KF), dtype=np.float32)
        u0[act, SRCL] = 1.0
        return g, u0

    xs = {}
    ys = {}
    acts = {}
    for j in range(2, M):
        xs[j], acts[j] = x_rows(j)
        ys[j] = y_rows(j)
    gb_, u0b = b_rows()

    def pack(gf, gbk):
        g = np.zeros((B, D, K2), dtype=np.float32)
        g[:, :, : gf.shape[2]] = gf
        g[:, :, KF : KF + gbk.shape[2]] = gbk
        return g

    P = [pack(ga, ys[2][0])]
    U = []
    u = np.zeros((B, K2), dtype=np.float32)
    u[:, START] = 1.0
    u[:, KF:] = ys[2][1]
    U.append(u)
    for j in range(2, M - 1):
        P.append(pack(xs[j], ys[j + 1][0]))
        u = np.zeros((B, K2), dtype=np.float32)
        u[acts[j], :K] = 1.0
        u[:, KF:] = ys[j + 1][1]
        U.append(u)
    P.append(pack(xs[M - 1], gb_))
    u = np.zeros((B, K2), dtype=np.float32)
    u[acts[M - 1], :K] = 1.0
    u[:, KF:] = u0b
    U.append(u)

    per_core = []
    for cix in range(NCORES):
        sl = slice(cix * BC, (cix + 1) * BC)
        gA = np.concatenate(
            [P[k][sl].transpose(2, 1, 0) for k in range(NG)], axis=2
        ).astype(ml_dtypes.float8_e4m3fn)
        gB = np.concatenate(
            [P[k][sl].transpose(2, 1, 0) for k in range(NG, NP)], axis=2
        ).astype(ml_dtypes.float8_e4m3fn)
        w0 = np.concatenate([U[k][sl].T for k in range(NP)], axis=1).astype(
            ml_dtypes.bfloat16
        )
        per_core.append(
            {
                "emisA": np.ascontiguousarray(gA),
                "emisB": np.ascontiguousarray(gB),
                "w0": np.ascontiguousarray(w0),
            }
        )

    S_ = np.zeros((K2, K2), dtype=np.float32)
    S_[:K, :K] = np.exp(trans).T
    S_[:K, HOLD] = estop32
    S_[HOLD, HOLD] = 1.0
    S_[KF : KF + K, KF : KF + K] = np.exp(trans)
    S_[KF + SRCL, KF : KF + K] = estop32
    S_[KF + SRCL, KF + SRCL] = 1.0
    etil = S_.astype(ml_dtypes.bfloat16)

    ar = np.arange(B)
    ctx = {
        "seq_len": L,
        "estop": estop,
        "C_at_L": Ccum[ar, L - 1],
        "Cal": Ccum[:, BND[1] - 1],
        "Cx": {j: Ccum[:, BND[j] - 1] - Ccum[:, BND[j - 1] - 1] for j in range(2, M)},
        "CyL": {j: Ccumb[ar, L - 1] - Ccumb[:, BND[j - 1] - 1] for j in range(2, M)},
        "Cb": Ccumb[ar, L - 1] - Ccumb[:, BND[M - 1] - 1],
    }
    return per_core, etil, ctx


def _combine(packs, ctx):
    """packs: list of NP arrays [K2, B] f64; returns per-batch scores."""
    L = ctx["seq_len"]
    estop = ctx["estop"]
    alpha = packs[0][:KF, :]
    xv = {}
    yv = {2: packs[0][KF : KF + K, :]}
    for j in range(2, M):
        xv[j] = packs[j - 1][:K, :]
    for j in range(2, M - 1):
        yv[j + 1] = packs[j - 1][KF : KF + K, :]
    bv = packs[NP - 1][KF : KF + K, :]

    d = {2: (yv[2] * alpha[:K, :]).sum(0)}
    n = {2: yv[2].sum(0)}
    for j in range(3, M):
        d[j] = (yv[j] * xv[j - 1]).sum(0)
        n[j] = yv[j].sum(0)
    d_b = (bv * xv[M - 1]).sum(0)

    scores = np.zeros(B)
    J1 = L <= BND[1] - 1
    scores[J1] = np.log(alpha[HOLD, J1]) + ctx["C_at_L"][J1]
    JM_ = L == BND[1]
    if JM_.any():
        dm = (alpha[:K, :] * estop[:, None]).sum(0)
        scores[JM_] = np.log(dm[JM_]) + ctx["Cal"][JM_]
    with np.errstate(divide="ignore", invalid="ignore"):
        for J in range(2, M):
            msk = (L > BND[J - 1]) & (L <= BND[J])
            sc = np.log(d[J]) + ctx["Cal"] + ctx["CyL"][J]
            for j in range(2, J):
                sc = sc + np.log(d[j]) - np.log(n[j]) + ctx["Cx"][j]
            scores[msk] = sc[msk]
        mskM = L > BND[M - 1]
        sc = np.log(d_b) + ctx["Cal"] + ctx["Cb"]
        for j in range(2, M):
            sc = sc + np.log(d[j]) - np.log(n[j]) + ctx["Cx"][j]
        scores[mskM] = sc[mskM]
    return scores


def _gold_score(feats, tags, seq_len, trans):
    feats = np.asarray(feats, dtype=np.float32)
    tags = np.asarray(tags, dtype=np.int64)
    seq_len = np.asarray(seq_len, dtype=np.int64)
    trans = np.asarray(trans, dtype=np.float32)
    tags_ext = np.concatenate(
        [np.full((B, 1), START, dtype=np.int64), tags], axis=1
    )
    trans_sc = trans[tags_ext[:, 1:], tags_ext[:, :-1]]
    emit_sc = np.take_along_axis(feats, tags_ext[:, 1:, None], axis=2)[..., 0]
    mask = np.arange(T)[None, :] < seq_len[:, None]
    last_tag = np.take_along_axis(tags_ext, seq_len[:, None], axis=1)[:, 0]
    gold = (
        np.where(mask, trans_sc + emit_sc, 0.0).sum(1, dtype=np.float64)
        + trans[STOP, last_tag]
    )
    return gold  # [B] f64


def kernel(feats, tags, seq_len, transitions):
    feats = np.asarray(feats)
    per_core, etil, ctx = _host_prep(feats, seq_len, transitions)
    nc = _build_module()
    in_maps = [{"etil": etil, **per_core[c]} for c in range(NCORES)]
    res = run_bass_kernel_spmd(nc, in_maps, list(range(NCORES)))
    outs = [np.asarray(res.results[c]["uout"]).astype(np.float64) for c in range(NCORES)]
    packs = [
        np.concatenate([o[:, k * BC : (k + 1) * BC] for o in outs], axis=1)
        for k in range(NP)
    ]
    scores = _combine(packs, ctx)
    gold = _gold_score(feats, tags, seq_len, transitions)
    loss = np.mean(scores - gold)
    return np.float32(loss)
